# revision 1
# baseline (speedup 1.0000x reference)
"""Trainium2 Bass kernel for nn_CausalSelfAttention_68496138437292.

Sharding: 8 cores = 2 batches x 4 head-groups. Core c handles batch b=c//4 and
heads [4*(c%4), 4*(c%4)+4). The Tversky projection is sharded over out_features
(each core computes a 256-wide o-slice); the feature contraction x_f (summed
over D, which is split across head-groups) uses one small AllReduce over each
batch's 4-core group.

All matmuls run in float32r (full-rate fp32 PE mode). The ternary weight
quantization (bf16 group-wise, matching the reference bit-exactly) and the
RoPE/YaRN tables are precomputed on the host; all heavy math (QKV projection,
rmsnorm stats, rope rotation, causal attention, softmax, Tversky projection)
runs on device.

Layout notes:
- q/k are computed transposed (d on partitions, s free). Matmul operands must
  start at base partition 0/32/64 and DVE ops cannot cross partitions, so the
  8 per-(head,half) 32-row blocks are packed 3-per-tile at offsets {0,32,64};
  projection PSUM is drained by DMA straight into that block layout.
- v lands [s, channel] groups of 33 (32 v columns + a ones column) so the PV
  matmul emits softmax denominators for free in PSUM row 32.
- Scores are computed transposed (S^T[ks, qs]); exp's per-partition ACT scale
  applies the k-side rmsnorm factor, the q-side factor (with q_gain/sqrt(d))
  is multiplied into q after rope via a DMA-broadcast tile.
- All partition-crossing moves (rope half-swap staging, reciprocal broadcast,
  attention-output writeback) go through DMA.
"""

import math
from contextlib import ExitStack

import ml_dtypes
import numpy as np

import concourse.bass as bass
import concourse.mybir as mybir
import concourse.tile as tile
from concourse import bacc
from concourse.bass_utils import run_bass_kernel_spmd

F32 = mybir.dt.float32
F32R = mybir.dt.float32r
AF = mybir.ActivationFunctionType
ALU = mybir.AluOpType

DIM, NH, HD = 1024, 16, 64
ROPE_BASE, TRAIN_LEN, YARN_MAX = 10000.0, 1024, 4096
GROUP = 64
EPS = 1e-05
B = 2
N_CORES = 8
HPC = 4          # heads per core
OSL = 256        # out-feature slice per core


# block layout: (head, half) -> (tile, partition offset); 3 blocks per tile
def _blk(h, f):
    idx = h * 2 + f
    return idx // 3, (idx % 3) * 32


_NBLK = [3, 3, 2]                      # valid blocks per q/k tile
_NROW = [96, 96, 64]                   # valid rows per q/k tile


# ----------------------------------------------------------------- host math

def _ternary_deq(w: np.ndarray) -> np.ndarray:
    """bf16 group-wise ternary dequantized weights; bit-exact with the jax
    reference (mean accumulates in f32, every elementwise op rounds to bf16)."""
    bf = ml_dtypes.bfloat16
    wb = w.astype(bf)
    wg = wb.reshape(-1, GROUP)
    m = (np.sum(np.abs(wg), axis=-1, keepdims=True, dtype=np.float32) / GROUP).astype(bf)
    scale = np.maximum(m.astype(np.float32), np.float32(1e-8)).astype(bf)
    ratio = (wg.astype(np.float32) / scale.astype(np.float32)).astype(bf)
    q = np.clip(np.round(ratio.astype(np.float32)), -1.0, 1.0).astype(bf)
    deq = (q.astype(np.float32) * scale.astype(np.float32)).astype(bf)
    return deq.reshape(wb.shape).astype(np.float32)


def _rope_tables(seqlen: int):
    rd = HD
    ar = np.arange(0, rd, 2, dtype=np.float32)
    inv_freq = 1.0 / ROPE_BASE ** (ar / rd)
    scale = TRAIN_LEN / YARN_MAX
    ramp = np.clip((ar / rd - 0.25) / 0.75, 0.0, 1.0)
    inv_freq = inv_freq / (ramp * (1.0 / scale - 1.0) + 1.0)
    freqs = np.arange(seqlen, dtype=np.float32)[:, None] * inv_freq[None, :]
    # [S, 32] -> [32, S]
    return np.cos(freqs).T.astype(np.float32), np.sin(freqs).T.astype(np.float32)


def _sigmoid(x):
    return 1.0 / (1.0 + np.exp(-x))


# ------------------------------------------------------------ device program

def build_program(S: int, lam: np.ndarray, dbg: bool = False):
    """Build the SPMD Bass program. lam: [HPC] per-local-head diff_lambda
    (baked as immediates; must be identical across head groups)."""
    NT = S // 128          # s-tiles
    NJ = S // 512          # 512-wide qs chunks
    SQ = S // 4            # projection s-quarter width
    NTQ = SQ // 128        # s-tiles per quarter

    nc = bacc.Bacc("TRN2", target_bir_lowering=False, debug=False,
                   num_devices=N_CORES)

    # DRAM I/O (per-core contents differ via in_maps)
    d_xT = nc.dram_tensor("xT", [DIM, S], F32R, kind="ExternalInput")
    d_wqkT = nc.dram_tensor("wqkT", [DIM, 512], F32R, kind="ExternalInput")
    d_wvT = nc.dram_tensor("wvT", [DIM, 256], F32R, kind="ExternalInput")
    d_cosF = nc.dram_tensor("cosF", [128, S], F32, kind="ExternalInput")
    d_sinF = nc.dram_tensor("sinF", [128, S], F32, kind="ExternalInput")
    d_bigmask = nc.dram_tensor("bigmask", [128, 896], F32, kind="ExternalInput")
    d_maskq = [nc.dram_tensor(f"maskq{t}", [128, 4], F32R, kind="ExternalInput")
               for t in range(3)]
    d_maskk = [nc.dram_tensor(f"maskk{t}", [128, 4], F32R, kind="ExternalInput")
               for t in range(3)]
    d_gains = nc.dram_tensor("gains", [4, 1], F32, kind="ExternalInput")
    d_featT = nc.dram_tensor("featT", [OSL, 16], F32R, kind="ExternalInput")
    d_AT = nc.dram_tensor("AT", [16, OSL], F32R, kind="ExternalInput")
    d_BT = nc.dram_tensor("BT", [16, OSL], F32R, kind="ExternalInput")
    d_out = nc.dram_tensor("out", [OSL, S], F32, kind="ExternalOutput")
    if dbg:
        d_dbg_qa = [nc.dram_tensor(f"dbg_qa{t}", [128, S], F32, kind="ExternalOutput")
                    for t in range(3)]
        d_dbg_ka = [nc.dram_tensor(f"dbg_ka{t}", [128, S], F32, kind="ExternalOutput")
                    for t in range(3)]
        d_dbg_v = nc.dram_tensor("dbg_v0", [128, 264], F32, kind="ExternalOutput")
        d_dbg_invT = nc.dram_tensor("dbg_invT", [128, 4 * (S // 128)], F32,
                                    kind="ExternalOutput")
        d_dbg_inva = nc.dram_tensor("dbg_inva", [4, S], F32, kind="ExternalOutput")
        d_dbg_yT = [nc.dram_tensor(f"dbg_yT{i}", [128, S], F32, kind="ExternalOutput")
                    for i in range(2)]
        d_dbg_xf = nc.dram_tensor("dbg_xf", [16, S], F32, kind="ExternalOutput")

    with tile.TileContext(nc) as tc:
        persist = ExitStack()
        cpool = persist.enter_context(tc.tile_pool(name="consts", bufs=1))
        vpool = persist.enter_context(tc.tile_pool(name="vstore", bufs=1))
        ivpool = persist.enter_context(tc.tile_pool(name="invT", bufs=1))
        ypool = persist.enter_context(tc.tile_pool(name="yT", bufs=1))
        qkpool = persist.enter_context(tc.tile_pool(name="qk", bufs=1))
        drpool = persist.enter_context(
            tc.tile_pool(name="drscratch", bufs=1, space="DRAM"))

        # small constants (live for the whole kernel)
        bigmask = cpool.tile([128, 896], F32, name="bigmask")
        maskq = [cpool.tile([128, 4], F32R, name=f"maskq{t}") for t in range(3)]
        maskk = [cpool.tile([128, 4], F32R, name=f"maskk{t}") for t in range(3)]
        gains = cpool.tile([4, 1], F32, name="gains")
        eps_col = cpool.tile([128, 1], F32, name="eps")
        zero_col = cpool.tile([128, 1], F32, name="zeroc")
        nc.vector.memset(zero_col[:], 0.0)
        featT = [cpool.tile([128, 16], F32R, name=f"featT{i}") for i in range(2)]
        AT = cpool.tile([16, OSL], F32R, name="AT")
        BT = cpool.tile([16, OSL], F32R, name="BT")
        nc.vector.memset(eps_col[:], EPS)
        nc.sync.dma_start(bigmask[:], d_bigmask[:])
        for t in range(3):
            nc.sync.dma_start(maskq[t][:], d_maskq[t][:])
            nc.sync.dma_start(maskk[t][:], d_maskk[t][:])
        nc.sync.dma_start(gains[:], d_gains[:])
        nc.sync.dma_start(featT[0][:], d_featT[0:128, :])
        nc.sync.dma_start(featT[1][:], d_featT[128:256, :])
        nc.sync.dma_start(AT[:], d_AT[:])
        nc.sync.dma_start(BT[:], d_BT[:])

        # v storage: per s-tile, 8 groups of (32 v-cols + ones col)
        v_store = [vpool.tile([128, 264], F32R, name=f"v{st}") for st in range(NT)]
        invT = ivpool.tile([128, 4 * NT], F32, name="invT")
        yT = [ypool.tile([128, S], F32R, name=f"yT{i}") for i in range(2)]
        # q/k in block layout; rope happens in place on these tiles
        qa = [qkpool.tile([128, S], F32R, name=f"qa{t}") for t in range(3)]
        ka = [qkpool.tile([128, S], F32R, name=f"ka{t}") for t in range(3)]

        # ---------------- phase 1: QKV projection ----------------
        proj_scope = ExitStack()
        wpool = proj_scope.enter_context(tc.tile_pool(name="weights", bufs=1))
        xpool = proj_scope.enter_context(tc.tile_pool(name="xstream", bufs=4))
        psq = proj_scope.enter_context(
            tc.tile_pool(name="psq", bufs=1, space="PSUM"))
        psv = proj_scope.enter_context(
            tc.tile_pool(name="psv", bufs=1, space="PSUM"))

        wqk = [wpool.tile([128, 512], F32R, name=f"wqk{d}") for d in range(8)]
        wv = [wpool.tile([128, 256], F32R, name=f"wv{d}") for d in range(8)]
        for d in range(8):
            nc.sync.dma_start(wqk[d][:], d_wqkT[d * 128:(d + 1) * 128, :])
            nc.sync.dma_start(wv[d][:], d_wvT[d * 128:(d + 1) * 128, :])

        for q4 in range(4):
            s0 = q4 * SQ
            pq = [psq.tile([128, SQ], F32, tag=f"pq{ch}", name=f"pq{ch}")
                  for ch in range(4)]
            pv = [psv.tile([128, 256], F32, tag=f"pv{st}", name=f"pv{st}")
                  for st in range(NTQ)]
            for d in range(8):
                xt = xpool.tile([128, SQ], F32R, tag="xt", name="xt")
                nc.sync.dma_start(xt[:], d_xT[d * 128:(d + 1) * 128, s0:s0 + SQ])
                for ch in range(4):
                    nc.tensor.matmul(
                        pq[ch][:], wqk[d][:, ch * 128:(ch + 1) * 128], xt[:],
                        start=(d == 0), stop=(d == 7))
                for st in range(NTQ):
                    nc.tensor.matmul(
                        pv[st][:], xt[:, st * 128:(st + 1) * 128], wv[d][:],
                        start=(d == 0), stop=(d == 7))
            # qk drain: ACT copy psum -> aligned SBUF temp, then DMA each
            # (head, half) 32-row block into the packed block layout.
            # psum ch-tile layout: 2 heads x 64.
            for ch in range(4):
                tmpd = xpool.tile([128, SQ], F32R, tag="tmpd", name="tmpd",
                                  bufs=4)
                nc.scalar.activation(tmpd[:], pq[ch][:], AF.Copy)
                dst_tiles = qa if ch < 2 else ka
                for hl in range(2):          # head within ch-tile
                    h = (ch % 2) * 2 + hl
                    for f in range(2):
                        t, o = _blk(h, f)
                        nc.sync.dma_start(
                            dst_tiles[t][o:o + 32, s0:s0 + SQ],
                            tmpd[hl * 64 + f * 32:hl * 64 + f * 32 + 32, :])
            for st in range(NTQ):
                gst = q4 * NTQ + st
                nc.vector.memset(v_store[gst][:].bitcast(F32), 1.0)
                dst = v_store[gst][:].rearrange("p (g c) -> p g c", c=33)[:, :, 0:32]
                src = pv[st][:].rearrange("p (g c) -> p g c", c=32)
                nc.scalar.activation(dst, src, AF.Copy)
        proj_scope.close()

        # ---------------- phase 2: rmsnorm stats ----------------
        qsc_scope = ExitStack()
        qsc_pool = qsc_scope.enter_context(tc.tile_pool(name="qscale", bufs=1))
        norm_scope = ExitStack()
        sqpool = norm_scope.enter_context(tc.tile_pool(name="sq", bufs=1))
        inva_pool = norm_scope.enter_context(tc.tile_pool(name="inva", bufs=1))
        psa = norm_scope.enter_context(
            tc.tile_pool(name="psa", bufs=1, space="PSUM"))
        psb = norm_scope.enter_context(
            tc.tile_pool(name="psb", bufs=3, space="PSUM"))

        # q-side: sumsq per (head, s) as [4, S]
        pa = psa.tile([4, S], F32, name="pa")
        for t in range(3):
            n = _NROW[t]
            sq = sqpool.tile([128, S], F32R, tag=f"sq{t}", name=f"sq{t}")
            nc.scalar.activation(sq[0:n, :], qa[t][0:n, :], AF.Square)
            for qc in range(S // 512):
                nc.tensor.matmul(
                    pa[:, qc * 512:(qc + 1) * 512], maskq[t][0:n, :],
                    sq[0:n, qc * 512:(qc + 1) * 512],
                    start=(t == 0), stop=(t == 2))
        inv_a = inva_pool.tile([4, S], F32, name="inva")
        nc.scalar.activation(inv_a[:], pa[:], AF.Sqrt, scale=1.0 / HD,
                             bias=eps_col[0:4, :])
        nc.vector.reciprocal(inv_a[:], inv_a[:])
        nc.vector.tensor_scalar_mul(inv_a[:], inv_a[:], gains[:])

        if dbg:
            nc.sync.dma_start(d_dbg_inva[:], inv_a[:])
        # broadcast inv_a rows into q-block layout via DRAM round trip
        dr_inva = drpool.tile([4, S], F32, name="dr_inva")
        nc.sync.dma_start(dr_inva[:], inv_a[:])
        qsc = [qsc_pool.tile([128, S], F32, name=f"qsc{t}") for t in range(3)]
        for h in range(4):
            for f in range(2):
                t, o = _blk(h, f)
                nc.sync.dma_start(
                    qsc[t][o:o + 32, :],
                    dr_inva[h:h + 1, :].to_broadcast([32, S]))

        # k-side: sumsq transposed as [s-tile, 4] columns of invT
        sq_k = []
        for t in range(3):
            n = _NROW[t]
            sq = sqpool.tile([128, S], F32R, tag=f"sqk{t}", name=f"sqk{t}")
            nc.scalar.activation(sq[0:n, :], ka[t][0:n, :], AF.Square)
            sq_k.append(sq)
        for st in range(NT):
            pb = psb.tile([128, 4], F32, tag="pb", name="pb")
            for t in range(3):
                n = _NROW[t]
                nc.tensor.matmul(
                    pb[:], sq_k[t][0:n, st * 128:(st + 1) * 128], maskk[t][0:n, :],
                    start=(t == 0), stop=(t == 2))
            nc.scalar.activation(invT[:, st * 4:st * 4 + 4], pb[:], AF.Sqrt,
                                 scale=1.0 / HD, bias=eps_col[:])
        nc.vector.reciprocal(invT[:], invT[:])
        norm_scope.close()

        # -------------- phase 3: rope (+ q scaling), in place --------------
        rope_scope = ExitStack()
        tripool = rope_scope.enter_context(tc.tile_pool(name="trig", bufs=1))
        rp = rope_scope.enter_context(tc.tile_pool(name="ropetmp", bufs=2))

        cosF = tripool.tile([128, S], F32, name="cosF")
        sinF = tripool.tile([128, S], F32, name="sinF")
        nc.sync.dma_start(cosF[:], d_cosF[:])
        nc.sync.dma_start(sinF[:], d_sinF[:])

        def rope_prefetch(tiles, tagp):
            # Prefetch ALL partner copies (other half of each head) before the
            # in-place overwrites: partner pairs can span tiles (h1's halves
            # live in tiles 0 and 1).
            prts = []
            for t in range(3):
                prt = rp.tile([128, S], F32R, tag=f"{tagp}{t}", name=f"{tagp}{t}", bufs=1)
                for k in range(_NBLK[t]):
                    idx = 3 * t + k
                    h, f = idx // 2, idx % 2
                    pt, po = _blk(h, 1 - f)
                    nc.sync.dma_start(prt[k * 32:k * 32 + 32, :],
                                      tiles[pt][po:po + 32, :])
                prts.append(prt)
            return prts

        def rope_apply(tiles, t, prt, qscale_tiles):
            # rot_f0 = x0*cos + x1*sin ; rot_f1 = x1*cos - x0*sin; sinF rows
            # carry the (+,-,+) f-parity sign of tiles 0/2, tile 1 subtracts.
            n = _NROW[t]
            tb = rp.tile([128, S], F32, tag="tb", name="tb")
            nc.vector.tensor_mul(tb[0:n, :], prt[0:n, :], sinF[0:n, :])
            nc.vector.tensor_mul(tiles[t][0:n, :], tiles[t][0:n, :],
                                 cosF[0:n, :])
            if t == 1:
                nc.vector.tensor_sub(tiles[t][0:n, :], tiles[t][0:n, :],
                                     tb[0:n, :])
            else:
                nc.vector.tensor_add(tiles[t][0:n, :], tiles[t][0:n, :],
                                     tb[0:n, :])
            if qscale_tiles is not None:
                nc.vector.tensor_mul(tiles[t][0:n, :], tiles[t][0:n, :],
                                     qscale_tiles[t][0:n, :])

        # tile 0 (heads 0/1) ropes first so attention can start while
        # tiles 1-2 are still rotating on DVE
        qprt = rope_prefetch(qa, "qp")
        kprt = rope_prefetch(ka, "kp")
        for t in range(3):
            rope_apply(qa, t, qprt[t], qsc)
            rope_apply(ka, t, kprt[t], None)
        rope_scope.close()
        qsc_scope.close()
        if dbg:
            for t in range(3):
                nc.sync.dma_start(d_dbg_qa[t][:], qa[t][:].bitcast(F32))
                nc.sync.dma_start(d_dbg_ka[t][:], ka[t][:].bitcast(F32))
            nc.sync.dma_start(d_dbg_v[:], v_store[0][:].bitcast(F32))
            nc.sync.dma_start(d_dbg_invT[:], invT[:])

        # ---------------- phase 4: attention ----------------
        attn_scope = ExitStack()
        epool = attn_scope.enter_context(tc.tile_pool(name="expS", bufs=24))
        tpool = attn_scope.enter_context(tc.tile_pool(name="exptmp", bufs=6))
        zpool = attn_scope.enter_context(tc.tile_pool(name="zc", bufs=10))
        rcpool = attn_scope.enter_context(tc.tile_pool(name="rcp", bufs=8))
        bcpool = attn_scope.enter_context(tc.tile_pool(name="bcast", bufs=8))
        pss = attn_scope.enter_context(
            tc.tile_pool(name="pss", bufs=5, space="PSUM"))
        psy = attn_scope.enter_context(
            tc.tile_pool(name="psy", bufs=3, space="PSUM"))

        for h in range(HPC):
            for j in range(NJ):
                ntk = 4 * (j + 1)
                LAG = 8
                es = {}
                py = {}
                for f in (0, 1):
                    py[f] = psy.tile([33, 512], F32, tag="py", name="py")

                def emit_pv(f, t):
                    g = 2 * h + f
                    off = max(0, (t - 4 * j) * 128)
                    nc.tensor.matmul(
                        py[f][:, off:512], v_store[t][:, g * 33:g * 33 + 33],
                        es.pop((f, t))[:, off:512],
                        start=(t == 0), stop=(t == ntk - 1))

                for t in range(ntk):
                    # causal boundary tiles only need columns >= off
                    off = max(0, (t - 4 * j) * 128)
                    w = 512 - off
                    for f in (0, 1):
                        qt, qo = _blk(h, f)
                        qr = qa[qt][qo:qo + 32,
                                    j * 512 + off:(j + 1) * 512]
                        ps = pss.tile([128, 512], F32, tag="ps", name="ps")
                        nc.tensor.matmul(
                            ps[:, off:512],
                            ka[qt][qo:qo + 32, t * 128:(t + 1) * 128],
                            qr, start=True, stop=True)
                        sc = invT[:, t * 4 + h:t * 4 + h + 1]
                        et = epool.tile([128, 512], F32R, tag="e", name="e")
                        if t >= 4 * j:  # boundary: exp valid cols, mask tri
                            tmp = tpool.tile([128, 512], F32, tag="tmp",
                                             name="tmp")
                            nc.scalar.activation(tmp[:, off:512],
                                                 ps[:, off:512], AF.Exp,
                                                 scale=sc)
                            nc.vector.tensor_mul(
                                et[:, off:512], tmp[:, off:512],
                                bigmask[:, 384:896 - off])
                        else:
                            nc.scalar.activation(et[:], ps[:], AF.Exp, scale=sc)
                        es[(f, t)] = et
                    if t >= LAG:
                        for f in (0, 1):
                            emit_pv(f, t - LAG)
                for t in range(max(0, ntk - LAG), ntk):
                    for f in (0, 1):
                        emit_pv(f, t)
                # combine halves (all at base partition 0, then DMA into yT)
                bb = {}
                for f in (0, 1):
                    rc = rcpool.tile([33, 512], F32, tag="rc", name="rc")
                    nc.vector.reciprocal(rc[32:33, :], py[f][32:33, :])
                    dr_rc = drpool.tile([1, 512], F32, tag="drrc", bufs=8,
                                        name="drrc")
                    nc.gpsimd.dma_start(dr_rc[:], rc[32:33, :])
                    bc = bcpool.tile([32, 512], F32, tag="bc", name="bc")
                    nc.gpsimd.dma_start(bc[:], dr_rc[:].to_broadcast([32, 512]))
                    bb[f] = bc
                z1 = zpool.tile([32, 512], F32, tag="z", name="z")
                z2 = zpool.tile([32, 512], F32, tag="z", name="z")
                zo1 = zpool.tile([32, 512], F32, tag="z", name="z")
                zo2 = zpool.tile([32, 512], F32, tag="z", name="z")
                nc.vector.tensor_mul(z1[:], py[0][0:32, :], bb[0][:])
                nc.vector.tensor_mul(z2[:], py[1][0:32, :], bb[1][:])
                lam_h = float(lam[h])
                nc.vector.scalar_tensor_tensor(
                    zo1[:], z2[:], -lam_h, z1[:], ALU.mult, ALU.add)
                nc.vector.scalar_tensor_tensor(
                    zo2[:], z2[:], lam_h, z1[:], ALU.mult, ALU.add)
                ti, r0 = h // 2, (h % 2) * 64
                nc.gpsimd.dma_start(
                    yT[ti][r0:r0 + 32, j * 512:(j + 1) * 512],
                    zo1[:].bitcast(F32R))
                nc.gpsimd.dma_start(
                    yT[ti][r0 + 32:r0 + 64, j * 512:(j + 1) * 512],
                    zo2[:].bitcast(F32R))
        attn_scope.close()

        # ---------------- phase 5: tversky projection ----------------
        tv_scope = ExitStack()
        xfpool = tv_scope.enter_context(tc.tile_pool(name="xf", bufs=1))
        psxf = tv_scope.enter_context(
            tc.tile_pool(name="psxf", bufs=1, space="PSUM"))

        NHALF = 1
        HS = S // NHALF
        xa, oms = [], []
        for half in range(NHALF):
            pxf = psxf.tile([16, HS], F32, tag=f"pxf{half}", name=f"pxf{half}")
            for qc2 in range(HS // 512):
                qc = half * (HS // 512) + qc2
                for dc in range(2):
                    nc.tensor.matmul(
                        pxf[:, qc2 * 512:(qc2 + 1) * 512], featT[dc][:],
                        yT[dc][:, qc * 512:(qc + 1) * 512],
                        start=(dc == 0), stop=(dc == 1))
            xf_loc = xfpool.tile([16, HS], F32, tag=f"xfl{half}",
                                 name=f"xfl{half}")
            nc.scalar.activation(xf_loc[:], pxf[:], AF.Copy)
            cc_in = drpool.tile([16, HS], F32, tag=f"ccin{half}",
                                name=f"ccin{half}")
            cc_out = drpool.tile([16, HS], F32, tag=f"ccout{half}",
                                 name=f"ccout{half}")
            nc.sync.dma_start(cc_in[:], xf_loc[:])
            nc.gpsimd.collective_compute(
                "AllReduce", ALU.add,
                replica_groups=[[0, 1, 2, 3], [4, 5, 6, 7]],
                ins=[cc_in[:]], outs=[cc_out[:]])
            xf = xfpool.tile([16, HS], F32, tag=f"xfr{half}",
                             name=f"xfr{half}")
            nc.sync.dma_start(xf[:], cc_out[:])
            xa_h = xfpool.tile([16, HS], F32R, tag=f"xa{half}",
                               name=f"xa{half}")
            oms_h = xfpool.tile([16, HS], F32R, tag=f"oms{half}",
                                name=f"oms{half}")
            nc.scalar.activation(xa_h[:], xf[:], AF.Silu, scale=5.0)
            nc.scalar.activation(oms_h[:], xf[:], AF.Sigmoid, scale=-5.0)
            xa.append(xa_h)
            oms.append(oms_h)

        tv2_scope = ExitStack()
        opool = tv2_scope.enter_context(tc.tile_pool(name="outsb", bufs=1))
        pso = tv2_scope.enter_context(
            tc.tile_pool(name="pso", bufs=2, space="PSUM"))

        out_sb = [opool.tile([128, S], F32, name=f"osb{i}") for i in range(2)]
        for ot in range(2):
            for qc in range(S // 512):
                po = pso.tile([128, 512], F32, tag="po", name="po")
                nc.tensor.matmul(
                    po[:], AT[:, ot * 128:(ot + 1) * 128],
                    xa[qc // (HS // 512)][:, (qc % (HS // 512)) * 512:(qc % (HS // 512) + 1) * 512], start=True, stop=False)
                nc.tensor.matmul(
                    po[:], BT[:, ot * 128:(ot + 1) * 128],
                    oms[qc // (HS // 512)][:, (qc % (HS // 512)) * 512:(qc % (HS // 512) + 1) * 512], start=False, stop=True)
                nc.scalar.activation(
                    out_sb[ot][:, qc * 512:(qc + 1) * 512], po[:], AF.Copy)
            nc.sync.dma_start(d_out[ot * 128:(ot + 1) * 128, :], out_sb[ot][:])
        tv2_scope.close()
        tv_scope.close()
        persist.close()

    nc.compile()
    return nc


# ----------------------------------------------------------- host marshaling

def make_in_maps(S, x, w_qkv, features, prototypes, theta, alpha, beta,
                 q_gain, diff_lambda):
    x = np.asarray(x, np.float32)
    w_qkv = np.asarray(w_qkv, np.float32)
    features = np.asarray(features, np.float32)
    prototypes = np.asarray(prototypes, np.float32)
    theta = float(np.abs(np.asarray(theta, np.float32)))
    alpha = float(np.abs(np.asarray(alpha, np.float32)))
    beta = float(np.abs(np.asarray(beta, np.float32)))
    q_gain = np.asarray(q_gain, np.float32)

    w_deq = _ternary_deq(w_qkv)
    p_deq = _ternary_deq(prototypes)
    cosT, sinT = _rope_tables(S)       # [32, S] each

    rows = np.arange(128)
    # rope tables in block layout: row r belongs to block r//32 with d = r%32.
    # sinF carries the sign of the f-parity pattern of tiles 0/2 (+,-,+);
    # tile 1's pattern (-,+,-) is realized by subtracting instead of adding.
    cosF = cosT[rows % 32, :]
    sgn = np.where((rows // 32) % 2 == 0, 1.0, -1.0).astype(np.float32)
    sinF = sinT[rows % 32, :] * sgn[:, None]

    bigmask = (np.arange(896)[None, :] >= rows[:, None] + 384).astype(np.float32)

    # norm masks in block layout: block k of tile t belongs to head (3t+k)//2
    masks = []
    for t in range(3):
        m = np.zeros((128, 4), np.float32)
        for k in range(_NBLK[t]):
            head = (3 * t + k) // 2
            m[k * 32:(k + 1) * 32, head] = 1.0
        masks.append(m)

    in_maps = []
    for c in range(N_CORES):
        b, hg = c // 4, c % 4
        h0 = hg * HPC
        qrows = slice(h0 * HD, (h0 + HPC) * HD)
        o0 = hg * OSL

        wqkT = np.ascontiguousarray(
            np.concatenate([w_deq[0:DIM][qrows],
                            w_deq[DIM:2 * DIM][qrows]], axis=0).T)
        wvT = np.ascontiguousarray(w_deq[2 * DIM:3 * DIM][qrows].T)
        xT = np.ascontiguousarray(x[b].T)

        gains = (q_gain[h0:h0 + HPC] / math.sqrt(HD // 2)).reshape(4, 1)
        featT = np.ascontiguousarray(features[:, o0:o0 + OSL].T)

        p_f = p_deq[o0:o0 + OSL] @ features.T          # [OSL, 16] f32
        p_s = _sigmoid(5.0 * p_f)
        p_a = p_f * p_s
        A_eff = (theta * p_a - alpha * (1.0 - p_s)) / 5.0
        B_eff = -beta * p_a
        m = {
            "xT": xT.astype(np.float32),
            "wqkT": wqkT.astype(np.float32),
            "wvT": wvT.astype(np.float32),
            "cosF": cosF, "sinF": sinF,
            "bigmask": bigmask,
            "gains": gains.astype(np.float32),
            "featT": featT.astype(np.float32),
            "AT": np.ascontiguousarray(A_eff.T).astype(np.float32),
            "BT": np.ascontiguousarray(B_eff.T).astype(np.float32),
        }
        for t in range(3):
            m[f"maskq{t}"] = masks[t]
            m[f"maskk{t}"] = masks[t]
        in_maps.append(m)
    return in_maps


def assemble_output(S, results):
    out = np.empty((B, S, DIM), np.float32)
    for c in range(N_CORES):
        b, hg = c // 4, c % 4
        out[b, :, hg * OSL:(hg + 1) * OSL] = results[c]["out"].T
    return out


_PROGRAM_CACHE = {}


def kernel(x, w_qkv, features, prototypes, theta, alpha, beta, q_gain,
           diff_lambda, _trace=False):
    x = np.asarray(x, np.float32)
    S = x.shape[1]
    lam = np.asarray(diff_lambda, np.float32)
    # lambdas are baked as immediates per local head; all 4 head groups share
    # one program, so they must agree across groups (true for these inputs).
    lam_local = lam.reshape(4, HPC)
    assert np.all(lam_local == lam_local[0:1]), "head-group-varying lambda"

    key = (S, lam_local[0].tobytes())
    if key not in _PROGRAM_CACHE:
        _PROGRAM_CACHE[key] = build_program(S, lam_local[0])
    nc = _PROGRAM_CACHE[key]

    in_maps = make_in_maps(S, x, w_qkv, features, prototypes, theta, alpha,
                           beta, q_gain, diff_lambda)
    res = run_bass_kernel_spmd(nc, in_maps, list(range(N_CORES)),
                               trace=_trace)
    out = assemble_output(S, res.results)
    if _trace:
        return out, res
    return out



# revision 22
# speedup vs baseline: 1.8311x; 1.8311x over previous
"""Trainium2 Bass kernel for nn_CausalSelfAttention_68496138437292.

Sharding: 8 cores = 2 batches x 4 head-groups; core c handles batch c//4 and
local heads [4*(c%4), 4*(c%4)+4).  The Tversky projection is sharded over
out_features (each core computes a 256-wide o-slice); the 16-wide feature
contraction x_f (summed over D, split across head-groups) uses one small
AllReduce per 512-token chunk over each batch's 4-core group.

Key structural ideas (vs the phase-serial f32r baseline):
- bf16 everywhere on the PE; all (head, half) channel blocks are packed 4 per
  128-partition tile in natural order, so projection PSUM drains straight into
  the attention layout with no DMA repacking.
- The attention OUTPUT is never materialized: the final output only needs
  x_f = feat . y^T, and  feat_slice . (P V / den)^T = (P (V W^T))^T / den,
  with  V W^T = x @ (w_v^T W^T)  folded into the projection as 128 extra
  channels (host-precomputed fold).  Attention per (head, half) reduces to
  scores -> exp -> one 17-wide PV matmul (16 feat cols + a ones column that
  emits the softmax denominator for free).
- Scores/PV use PE array tiling (tile_position) for ~1.4x matmul throughput;
  exp runs as [128, 1024] ACT ops over multi-bank PSUM with trimmed causal
  widths; rmsnorm rsqrt+gain is one ln + one exp (single ACT table set).
- Projection of chunk c+1 is interleaved into attention of chunk j=c so the
  PE keeps busy while ACT chews exp; Tversky tail is pipelined per chunk with
  one [16, 512] AllReduce each.
"""

import math
from contextlib import ExitStack

import ml_dtypes
import numpy as np

import concourse.bass as bass
import concourse.mybir as mybir
import concourse.tile as tile
from concourse import bacc
from concourse.bass_utils import run_bass_kernel_spmd

F32 = mybir.dt.float32
F32R = mybir.dt.float32r
BF16 = mybir.dt.bfloat16
AF = mybir.ActivationFunctionType
ALU = mybir.AluOpType

DIM, NH, HD = 1024, 16, 64
ROPE_BASE, TRAIN_LEN, YARN_MAX = 10000.0, 1024, 4096
GROUP = 64
EPS = 1e-05
B = 2
N_CORES = 8
HPC = 4          # heads per core
OSL = 256        # out-feature slice per core
NF = 16          # tversky feature count
PVW = 17         # PV rhs width: 16 feat cols + ones col

BF = ml_dtypes.bfloat16


# ----------------------------------------------------------------- host math

def _ternary_deq(w: np.ndarray) -> np.ndarray:
    bf = ml_dtypes.bfloat16
    wb = w.astype(bf)
    wg = wb.reshape(-1, GROUP)
    m = (np.sum(np.abs(wg), axis=-1, keepdims=True, dtype=np.float32) / GROUP).astype(bf)
    scale = np.maximum(m.astype(np.float32), np.float32(1e-8)).astype(bf)
    ratio = (wg.astype(np.float32) / scale.astype(np.float32)).astype(bf)
    q = np.clip(np.round(ratio.astype(np.float32)), -1.0, 1.0).astype(bf)
    deq = (q.astype(np.float32) * scale.astype(np.float32)).astype(bf)
    return deq.reshape(wb.shape).astype(np.float32)


def _rope_tables(seqlen: int):
    rd = HD
    ar = np.arange(0, rd, 2, dtype=np.float32)
    inv_freq = 1.0 / ROPE_BASE ** (ar / rd)
    scale = TRAIN_LEN / YARN_MAX
    ramp = np.clip((ar / rd - 0.25) / 0.75, 0.0, 1.0)
    inv_freq = inv_freq / (ramp * (1.0 / scale - 1.0) + 1.0)
    freqs = np.arange(seqlen, dtype=np.float32)[:, None] * inv_freq[None, :]
    return np.cos(freqs).T.astype(np.float32), np.sin(freqs).T.astype(np.float32)


def _sigmoid(x):
    return 1.0 / (1.0 + np.exp(-x))


# ------------------------------------------------------------ device program

def build_program(S: int, dbg: bool = False):
    NC = S // 512            # 512-token chunks
    NT = S // 128            # 128-token k tiles

    nc = bacc.Bacc("TRN2", target_bir_lowering=False, debug=False,
                   num_devices=N_CORES)

    # DRAM I/O
    d_xT = nc.dram_tensor("xT", [DIM, S], BF16, kind="ExternalInput")
    # 640 = 256 q + 256 k + 128 vw-fold channels
    d_wT = nc.dram_tensor("wT", [DIM, 640], BF16, kind="ExternalInput")
    d_cosF = nc.dram_tensor("cosF", [128, S], BF16, kind="ExternalInput")
    d_sinF = nc.dram_tensor("sinF", [128, S], BF16, kind="ExternalInput")
    d_tri4 = nc.dram_tensor("tri4", [128, 512], BF16, kind="ExternalInput")
    d_mask8 = nc.dram_tensor("mask8", [128, 32], BF16, kind="ExternalInput")
    d_smask = nc.dram_tensor("smask", [128, 16], BF16, kind="ExternalInput")
    d_lngain = nc.dram_tensor("lngain", [8, 1], F32, kind="ExternalInput")
    d_ident = nc.dram_tensor("ident", [128, 128], BF16, kind="ExternalInput")
    d_AT = nc.dram_tensor("AT", [16, OSL], BF16, kind="ExternalInput")
    d_BT = nc.dram_tensor("BT", [16, OSL], BF16, kind="ExternalInput")
    d_out = nc.dram_tensor("out", [OSL, S], F32, kind="ExternalOutput")
    if dbg:
        d_dbg_xf = nc.dram_tensor("dbg_xf", [16, S], F32, kind="ExternalOutput")
        d_dbg_xfar = nc.dram_tensor("dbg_xfar", [16, S], F32, kind="ExternalOutput")

    with tile.TileContext(nc) as tc:
        persist = ExitStack()
        cpool = persist.enter_context(tc.tile_pool(name="consts", bufs=1))
        qkpool = persist.enter_context(tc.tile_pool(name="qk", bufs=1))
        vwpool = persist.enter_context(tc.tile_pool(name="vwrhs", bufs=1))
        xfpool = persist.enter_context(tc.tile_pool(name="xft", bufs=1))
        wpool = persist.enter_context(tc.tile_pool(name="wts", bufs=1))
        drpool = persist.enter_context(
            tc.tile_pool(name="drscratch", bufs=1, space="DRAM"))

        # transient pools
        xpool = persist.enter_context(tc.tile_pool(name="xstream", bufs=8))
        sqpool = persist.enter_context(tc.tile_pool(name="sq", bufs=4))
        scpool = persist.enter_context(tc.tile_pool(name="scb", bufs=4))
        s8pool = persist.enter_context(tc.tile_pool(name="sc8", bufs=2))
        rppool = persist.enter_context(tc.tile_pool(name="ropetmp", bufs=2))
        espool = persist.enter_context(tc.tile_pool(name="es", bufs=6))
        tailp = persist.enter_context(tc.tile_pool(name="tail", bufs=2))

        # PSUM budget (8 banks): proj 1 + aux 1 + waves 2x2 + pv 2 = 8
        proj_ps = persist.enter_context(
            tc.tile_pool(name="proj", bufs=1, space="PSUM"))
        aux_ps = persist.enter_context(
            tc.tile_pool(name="aux", bufs=1, space="PSUM"))
        wave_ps = persist.enter_context(
            tc.tile_pool(name="wave", bufs=2, space="PSUM"))
        pv_ps = persist.enter_context(
            tc.tile_pool(name="pv", bufs=1, space="PSUM"))

        def aux_tile():
            # single shared [128, 512] psum bank; callers slice what they need
            return aux_ps.tile([128, 512], F32, tag="aux", name="aux")

        # ---- persistent SBUF ----
        cosF = cpool.tile([128, S], BF16, name="cosF")
        sinF = cpool.tile([128, S], BF16, name="sinF")
        tri4 = cpool.tile([128, 512], BF16, name="tri4")
        mask8 = cpool.tile([128, 32], BF16, name="mask8")
        smask = cpool.tile([128, 16], BF16, name="smask")
        lngain = cpool.tile([8, 1], F32, name="lngain")
        eps8 = cpool.tile([8, 1], F32, name="eps8")
        nc.vector.memset(eps8[:], EPS)
        ident = cpool.tile([128, 128], BF16, name="ident")
        AT = cpool.tile([16, OSL], BF16, name="AT")
        BT = cpool.tile([16, OSL], BF16, name="BT")
        nc.sync.dma_start(cosF[:], d_cosF[:])
        nc.sync.dma_start(sinF[:], d_sinF[:])
        nc.sync.dma_start(tri4[:], d_tri4[:])
        nc.sync.dma_start(mask8[:], d_mask8[:])
        nc.sync.dma_start(smask[:], d_smask[:])
        nc.sync.dma_start(lngain[:], d_lngain[:])
        nc.sync.dma_start(ident[:], d_ident[:])
        nc.sync.dma_start(AT[:], d_AT[:])
        nc.sync.dma_start(BT[:], d_BT[:])

        wts = [wpool.tile([128, 640], BF16, name=f"w{d}") for d in range(8)]
        for d in range(8):
            nc.sync.dma_start(wts[d][:], d_wT[d * 128:(d + 1) * 128, :])

        qa = [qkpool.tile([128, S], BF16, name=f"qa{t}") for t in range(2)]
        ka = [qkpool.tile([128, S], BF16, name=f"ka{t}") for t in range(2)]
        # PV rhs per ktile: [k 128, 8 problems x 17]; ones col at 16 mod 17
        rhs_vw = [vwpool.tile([128, 8 * PVW], BF16, name=f"rvw{t}")
                  for t in range(NT)]
        for t in range(NT):
            nc.vector.memset(
                rhs_vw[t][:].rearrange("p (g c) -> p g c", c=PVW)[:, :, 16:17],
                1.0)
        # xf^T strips per tile-group: rows 32p..32p+16 = (xf contrib | den)
        xft = [xfpool.tile([128, S], F32, name=f"xft{t}") for t in range(2)]

        # ---------------- emission helpers ----------------

        def proj_chunk_steps(c):
            """Returns a list of closures emitting projection of chunk c."""
            s0 = c * 512
            steps = []
            xt = [None] * 8
            sq_t = [None] * 4
            stat = [None]
            sc8 = [None]

            def load_x():
                for d in range(8):
                    xt[d] = xpool.tile([128, 512], BF16, tag="xt", name="xt")
                    nc.sync.dma_start(xt[d][:], d_xT[d * 128:(d + 1) * 128,
                                                     s0:s0 + 512])
            steps.append(load_x)

            # 4 qk chains: ot 0,1 = q tiles, ot 2,3 = k tiles
            def make_qk(ot):
                def f():
                    dst = qa[ot] if ot < 2 else ka[ot - 2]
                    pq = proj_ps.tile([128, 512], F32, tag="pmm", name="pmm")
                    for d in range(8):
                        nc.tensor.matmul(pq[:], wts[d][:, ot * 128:(ot + 1) * 128],
                                         xt[d][:], start=(d == 0), stop=(d == 7))
                    # unscaled drain (scale applied later in-place)
                    nc.vector.tensor_copy(dst[:, s0:s0 + 512], pq[:])
                    sq = sqpool.tile([128, 512], BF16, tag="sq", name="sq")
                    nc.vector.tensor_mul(sq[:], dst[:, s0:s0 + 512],
                                         dst[:, s0:s0 + 512])
                    sq_t[ot] = sq
                return f
            for ot in range(4):
                steps.append(make_qk(ot))

            def stats():
                st = aux_tile()
                for ot in range(4):
                    nc.tensor.matmul(st[0:8, :], mask8[:, ot * 8:(ot + 1) * 8],
                                     sq_t[ot][:], start=(ot == 0), stop=(ot == 3))
                lnt = s8pool.tile([8, 512], F32, tag="lnt", name="lnt")
                nc.scalar.activation(lnt[:], st[0:8, :], AF.Ln, scale=1.0 / HD,
                                     bias=eps8[:])
                s8 = s8pool.tile([8, 512], BF16, tag="sc8", name="sc8")
                nc.scalar.activation(s8[:], lnt[:], AF.Exp, scale=-0.5,
                                     bias=lngain[:])
                sc8[0] = s8
            steps.append(stats)

            def scale_rope():
                # broadcast row scales into block layout (via DRAM scratch --
                # SBUF sources cannot have a zero partition step), then
                # scale+rope
                dr8 = drpool.tile([8, 512], BF16, tag="dr8", bufs=2, name="dr8")
                nc.sync.dma_start(dr8[:], sc8[0][:])
                scb = []
                for tt in range(4):          # 2 q tiles then 2 k tiles
                    sb = scpool.tile([128, 512], BF16, tag="scb", name="scb")
                    for hh in range(2):
                        row = (tt // 2) * 4 + (tt % 2) * 2 + hh
                        nc.gpsimd.dma_start(
                            sb[hh * 64:hh * 64 + 64, :],
                            dr8[row:row + 1, :].to_broadcast([64, 512]))
                    scb.append(sb)
                tiles = [qa[0], qa[1], ka[0], ka[1]]
                for tt in range(4):
                    nc.vector.tensor_mul(tiles[tt][:, s0:s0 + 512],
                                         tiles[tt][:, s0:s0 + 512], scb[tt][:])
                # rope: prefetch partner blocks (adjacent 32-row block), then
                # x = x*cos + prt*sinF (sinF carries the half sign)
                for tt in range(4):
                    prt = rppool.tile([128, 512], BF16, tag="prt", name="prt")
                    for p in range(4):
                        nc.sync.dma_start(
                            prt[32 * p:32 * p + 32, :],
                            tiles[tt][32 * (p ^ 1):32 * (p ^ 1) + 32, s0:s0 + 512])
                    tb = rppool.tile([128, 512], BF16, tag="tb", name="tb")
                    nc.vector.tensor_mul(tb[:], prt[:], sinF[:, s0:s0 + 512])
                    nc.vector.tensor_mul(tiles[tt][:, s0:s0 + 512],
                                         tiles[tt][:, s0:s0 + 512],
                                         cosF[:, s0:s0 + 512])
                    nc.vector.tensor_add(tiles[tt][:, s0:s0 + 512],
                                         tiles[tt][:, s0:s0 + 512], tb[:])
            steps.append(scale_rope)

            def vw_chain():
                pv = proj_ps.tile([128, 512], F32, tag="pmm", name="pmm")
                for d in range(8):
                    nc.tensor.matmul(pv[:], wts[d][:, 512:640], xt[d][:],
                                     start=(d == 0), stop=(d == 7))
                vw_sb = sqpool.tile([128, 512], BF16, tag="vwsb", name="vwsb")
                nc.vector.tensor_copy(vw_sb[:], pv[:])
                # transpose each 128-token block: vw_sb [ch 128, s] -> [s, ch]
                for i in range(4):
                    pt = aux_tile()
                    nc.tensor.matmul(pt[:, 0:128], vw_sb[:, i * 128:(i + 1) * 128],
                                     ident[:], start=True, stop=True)
                    t = c * 4 + i
                    dst = rhs_vw[t][:].rearrange(
                        "p (g c) -> p g c", c=PVW)[:, :, 0:16]
                    nc.vector.tensor_copy(
                        dst, pt[:, 0:128].rearrange("p (g c) -> p g c", c=16))
            steps.append(vw_chain)
            return steps

        # attention state: per chunk j, accumulate xf strips in 2 psum banks
        def attn_chunk(j, inject):
            """Emit attention for q chunk j; call inject() between t-steps to
            interleave next chunk's projection work."""
            ntk = 4 * (j + 1)
            xfa = [pv_ps.tile([128, 512], F32, tag=f"xfa{qt}", name=f"xfa{qt}")
                   for qt in range(2)]
            es_q = {}
            LAG = 2

            def emit_pv(t):
                off = max(0, (t - 4 * j) * 128)
                for qt in range(2):
                    es = es_q.pop((t, qt))
                    es3 = es[:].rearrange("p (g c) -> p g c", c=512)
                    for p in range(4):
                        nc.tensor.matmul(
                            xfa[qt][32 * p:32 * p + PVW, off:512],
                            rhs_vw[t][:, (4 * qt + p) * PVW:(4 * qt + p + 1) * PVW],
                            es3[:, p, off:512],
                            start=(t == 0), stop=(t == ntk - 1),
                            tile_position=(0, 32 * p),
                            skip_group_check=True)

            for t in range(ntk):
                off = max(0, (t - 4 * j) * 128)
                w = 512 - off
                for qt in range(2):
                    es = espool.tile([128, 2048], BF16, tag="es", name="es")
                    for pair in range(2):
                        ps = wave_ps.tile([128, 1024], F32, tag="wv", name="wv")
                        for pp in range(2):
                            p = pair * 2 + pp
                            nc.tensor.matmul(
                                ps[:, pp * 512 + off:(pp + 1) * 512],
                                ka[qt][32 * p:32 * p + 32, t * 128:(t + 1) * 128],
                                qa[qt][32 * p:32 * p + 32,
                                       j * 512 + off:(j + 1) * 512],
                                start=True, stop=True,
                                tile_position=(32 * p, 0))
                        ps3 = ps[:].rearrange("p (g c) -> p g c", c=512)
                        es3 = es[:].rearrange("p (g c) -> p g c", c=512)
                        nc.scalar.activation(
                            es3[:, 2 * pair:2 * pair + 2, off:512],
                            ps3[:, :, off:512], AF.Exp)
                    if t >= 4 * j:
                        # causal mask on the diagonal 128-block of each strip
                        dv = es[:].rearrange("p (g c) -> p g c", c=512)[
                            :, :, off:off + 128]
                        tri = tri4[:].rearrange("p (g c) -> p g c", c=128)
                        nc.vector.tensor_mul(dv, dv, tri)
                    es_q[(t, qt)] = es
                if t >= LAG:
                    emit_pv(t - LAG)
                inject()
            for t in range(max(0, ntk - LAG), ntk):
                emit_pv(t)

            # drain strips to SBUF
            for qt in range(2):
                nc.vector.tensor_copy(xft[qt][:, j * 512:(j + 1) * 512],
                                      xfa[qt][:])

        # ---------------- tversky tail (per chunk) ----------------
        cc_in = [drpool.tile([16, 512], F32, name=f"ccin{j}") for j in range(NC)]
        cc_out = [drpool.tile([16, 512], F32, name=f"ccout{j}") for j in range(NC)]

        def tail_chunk(j):
            s0 = j * 512
            # gather dens rows (strip row 16 of each 32-block), reciprocal,
            # broadcast back over 16 rows of each strip
            dens = tailp.tile([8, 512], F32, tag="dens", name="dens")
            for qt in range(2):
                for p in range(4):
                    nc.gpsimd.dma_start(
                        dens[qt * 4 + p:qt * 4 + p + 1, :],
                        xft[qt][32 * p + 16:32 * p + 17, s0:s0 + 512])
            rb = tailp.tile([8, 512], F32, tag="rb", name="rb")
            nc.vector.reciprocal(rb[:], dens[:])
            drb = drpool.tile([8, 512], F32, tag="drb", bufs=2, name="drb")
            nc.sync.dma_start(drb[:], rb[:])
            rbb = [tailp.tile([128, 512], F32, tag=f"rbb{qt}", name="rbb")
                   for qt in range(2)]
            for qt in range(2):
                for p in range(4):
                    nc.gpsimd.dma_start(
                        rbb[qt][32 * p:32 * p + 16, :],
                        drb[qt * 4 + p:qt * 4 + p + 1, :].to_broadcast([16, 512]))
            sc = [tailp.tile([128, 512], BF16, tag=f"sc{qt}", name="sc")
                  for qt in range(2)]
            for qt in range(2):
                nc.vector.tensor_mul(sc[qt][:],
                                     xft[qt][:, s0:s0 + 512], rbb[qt][:])
            pxf = aux_tile()
            for qt in range(2):
                nc.tensor.matmul(pxf[0:16, :], smask[:], sc[qt][:],
                                 start=(qt == 0), stop=(qt == 1))
            xfl = tailp.tile([16, 512], F32, tag="xfl", name="xfl")
            nc.vector.tensor_copy(xfl[:], pxf[0:16, :])
            if dbg:
                nc.sync.dma_start(d_dbg_xf[:, s0:s0 + 512], xfl[:])
            nc.sync.dma_start(cc_in[j][:], xfl[:])
            nc.gpsimd.collective_compute(
                "AllReduce", ALU.add,
                replica_groups=[[0, 1, 2, 3], [4, 5, 6, 7]],
                ins=[cc_in[j][:]], outs=[cc_out[j][:]])
            xf = tailp.tile([16, 512], F32, tag="xfr", name="xfr")
            nc.sync.dma_start(xf[:], cc_out[j][:])
            if dbg:
                nc.sync.dma_start(d_dbg_xfar[:, s0:s0 + 512], xf[:])
            # sigmoid path: xa = xf*sig(5xf) = xf/(1+e), oms = e/(1+e), e=exp(-5xf)
            e = tailp.tile([16, 512], F32, tag="e", name="e")
            nc.scalar.activation(e[:], xf[:], AF.Exp, scale=-5.0)
            t1 = tailp.tile([16, 512], F32, tag="t1", name="t1")
            nc.vector.tensor_scalar_add(t1[:], e[:], 1.0)
            r = tailp.tile([16, 512], F32, tag="r", name="r")
            nc.vector.reciprocal(r[:], t1[:])
            xa = tailp.tile([16, 512], BF16, tag="xa", name="xa")
            oms = tailp.tile([16, 512], BF16, tag="oms", name="oms")
            nc.vector.tensor_mul(xa[:], xf[:], r[:])
            nc.vector.tensor_mul(oms[:], e[:], r[:])
            # out projection: out[o, s] = AT^T xa + BT^T oms
            for ot in range(2):
                po = aux_tile()
                nc.tensor.matmul(po[:], AT[:, ot * 128:(ot + 1) * 128], xa[:],
                                 start=True, stop=False)
                nc.tensor.matmul(po[:], BT[:, ot * 128:(ot + 1) * 128], oms[:],
                                 start=False, stop=True)
                ob = tailp.tile([128, 512], F32, tag="ob", name="ob")
                nc.vector.tensor_copy(ob[:], po[:])
                nc.sync.dma_start(d_out[ot * 128:(ot + 1) * 128, s0:s0 + 512],
                                  ob[:])

        # ---------------- main schedule ----------------
        for step in proj_chunk_steps(0):
            step()
        pending = []
        for j in range(NC):
            if j + 1 < NC:
                pending = proj_chunk_steps(j + 1)
            else:
                pending = []
            # spread pending proj steps across this chunk's t loop
            counter = [0]
            total_t = 4 * (j + 1)
            nsteps = len(pending)

            def inject():
                counter[0] += 1
                want = (counter[0] * nsteps) // total_t
                while len(pending) and (nsteps - len(pending)) < want:
                    pending.pop(0)()
            attn_chunk(j, inject)
            while pending:
                pending.pop(0)()
            tail_chunk(j)

        persist.close()

    nc.compile()
    return nc


# ----------------------------------------------------------- host marshaling

def make_in_maps(S, x, w_qkv, features, prototypes, theta, alpha, beta,
                 q_gain, diff_lambda):
    x = np.asarray(x, np.float32)
    w_qkv = np.asarray(w_qkv, np.float32)
    features = np.asarray(features, np.float32)
    prototypes = np.asarray(prototypes, np.float32)
    theta = float(np.abs(np.asarray(theta, np.float32)))
    alpha = float(np.abs(np.asarray(alpha, np.float32)))
    beta = float(np.abs(np.asarray(beta, np.float32)))
    q_gain = np.asarray(q_gain, np.float32)
    lam = np.asarray(diff_lambda, np.float32)

    w_deq = _ternary_deq(w_qkv)
    p_deq = _ternary_deq(prototypes)
    cosT, sinT = _rope_tables(S)       # [32, S]

    rows = np.arange(128)
    sgn = np.where((rows // 32) % 2 == 0, 1.0, -1.0).astype(np.float32)
    cosF = cosT[rows % 32, :].astype(BF)
    sinF = (sinT[rows % 32, :] * sgn[:, None]).astype(BF)

    # diag-block causal mask, tiled 4x horizontally: [128, 512]
    tri = (np.arange(128)[None, :] >= np.arange(128)[:, None]).astype(np.float32)
    tri4 = np.tile(tri, (1, 4)).astype(BF)

    # stats masks: [128, 32] = 4 tile-types x 8 stat rows
    mask8 = np.zeros((128, 32), np.float32)
    for tt in range(4):
        for r in range(128):
            head_in_tile = r // 64
            row = (tt // 2) * 4 + (tt % 2) * 2 + head_in_tile
            mask8[r, tt * 8 + row] = 1.0
    mask8 = mask8.astype(BF)

    # strip-sum mask: rows 32p+i (i<16) -> col i
    smask = np.zeros((128, 16), np.float32)
    for p in range(4):
        for i in range(16):
            smask[32 * p + i, i] = 1.0

    ident = np.eye(128, dtype=np.float32).astype(BF)

    in_maps = []
    for c in range(N_CORES):
        b, hg = c // 4, c % 4
        h0 = hg * HPC
        qrows = slice(h0 * HD, (h0 + HPC) * HD)
        o0 = hg * OSL

        wq = w_deq[0:DIM][qrows]                   # [256, 1024]
        wk = w_deq[DIM:2 * DIM][qrows]             # [256, 1024]
        wv = w_deq[2 * DIM:3 * DIM][qrows]         # [256, 1024]

        # vw fold: per (h, f) channel block [k,16] = x @ (wv_f^T W_hf^T)
        # W_h0 = M1 + M2, W_h1 = lam_h (M2 - M1), M = features[:, head dims]
        wfold = np.zeros((DIM, 8 * NF), np.float32)
        for h in range(HPC):
            gh = h0 + h
            M1 = features[:, gh * 64:gh * 64 + 32]       # [16, 32]
            M2 = features[:, gh * 64 + 32:gh * 64 + 64]
            Wh0 = (M1 + M2)                               # [16, 32]
            Wh1 = lam[gh] * (M2 - M1)
            v0 = wv[h * 64:h * 64 + 32]                   # [32, 1024]
            v1 = wv[h * 64 + 32:h * 64 + 64]
            wfold[:, (2 * h) * NF:(2 * h + 1) * NF] = v0.T @ Wh0.T
            wfold[:, (2 * h + 1) * NF:(2 * h + 2) * NF] = v1.T @ Wh1.T

        wT = np.concatenate([wq.T, wk.T, wfold], axis=1)  # [1024, 640]

        gains = q_gain[h0:h0 + HPC] / math.sqrt(HD // 2)
        assert np.all(gains > 0), "nonpositive q_gain unsupported by ln-fold"
        lngain = np.zeros((8, 1), np.float32)
        lngain[0:4, 0] = np.log(gains)

        p_f = p_deq[o0:o0 + OSL] @ features.T          # [256, 16]
        p_s = _sigmoid(5.0 * p_f)
        p_a = p_f * p_s
        A_eff = theta * p_a - alpha * (1.0 - p_s)
        B_eff = -beta * p_a

        m = {
            "xT": np.ascontiguousarray(x[b].T).astype(BF),
            "wT": np.ascontiguousarray(wT).astype(BF),
            "cosF": cosF, "sinF": sinF,
            "tri4": tri4, "mask8": mask8,
            "smask": smask.astype(BF),
            "lngain": lngain,
            "ident": ident,
            "AT": np.ascontiguousarray(A_eff.T).astype(BF),
            "BT": np.ascontiguousarray(B_eff.T).astype(BF),
        }
        in_maps.append(m)
    return in_maps


def assemble_output(S, results):
    out = np.empty((B, S, DIM), np.float32)
    for c in range(N_CORES):
        b, hg = c // 4, c % 4
        out[b, :, hg * OSL:(hg + 1) * OSL] = results[c]["out"].T
    return out


_PROGRAM_CACHE = {}


def kernel(x, w_qkv, features, prototypes, theta, alpha, beta, q_gain,
           diff_lambda, _trace=False):
    x = np.asarray(x, np.float32)
    S = x.shape[1]
    if S not in _PROGRAM_CACHE:
        _PROGRAM_CACHE[S] = build_program(S)
    nc = _PROGRAM_CACHE[S]

    in_maps = make_in_maps(S, x, w_qkv, features, prototypes, theta, alpha,
                           beta, q_gain, diff_lambda)
    res = run_bass_kernel_spmd(nc, in_maps, list(range(N_CORES)),
                               trace=_trace)
    out = assemble_output(S, res.results)
    if _trace:
        return out, res
    return out


# revision 25
# speedup vs baseline: 1.9098x; 1.0430x over previous
"""Trainium2 Bass kernel for nn_CausalSelfAttention_68496138437292.

Sharding: 8 cores = 2 batches x 4 head-groups; core c handles batch c//4 and
local heads [4*(c%4), 4*(c%4)+4).  The Tversky projection is sharded over
out_features (each core computes a 256-wide o-slice); the 16-wide feature
contraction x_f (summed over D, split across head-groups) uses one small
AllReduce per 512-token chunk over each batch's 4-core group.

Key structural ideas (vs the phase-serial f32r baseline):
- bf16 everywhere on the PE; all (head, half) channel blocks are packed 4 per
  128-partition tile in natural order, so projection PSUM drains straight into
  the attention layout with no DMA repacking.
- The attention OUTPUT is never materialized: the final output only needs
  x_f = feat . y^T, and  feat_slice . (P V / den)^T = (P (V W^T))^T / den,
  with  V W^T = x @ (w_v^T W^T)  folded into the projection as 128 extra
  channels (host-precomputed fold).  Attention per (head, half) reduces to
  scores -> exp -> one 17-wide PV matmul (16 feat cols + a ones column that
  emits the softmax denominator for free).
- Scores/PV use PE array tiling (tile_position) for ~1.4x matmul throughput;
  exp runs as [128, 1024] ACT ops over multi-bank PSUM with trimmed causal
  widths; rmsnorm rsqrt+gain is one ln + one exp (single ACT table set).
- Projection of chunk c+1 is interleaved into attention of chunk j=c so the
  PE keeps busy while ACT chews exp; Tversky tail is pipelined per chunk with
  one [16, 512] AllReduce each.
"""

import math
from contextlib import ExitStack

import ml_dtypes
import numpy as np

import concourse.bass as bass
import concourse.mybir as mybir
import concourse.tile as tile
from concourse import bacc
from concourse.bass_utils import run_bass_kernel_spmd

F32 = mybir.dt.float32
F32R = mybir.dt.float32r
BF16 = mybir.dt.bfloat16
AF = mybir.ActivationFunctionType
ALU = mybir.AluOpType

DIM, NH, HD = 1024, 16, 64
ROPE_BASE, TRAIN_LEN, YARN_MAX = 10000.0, 1024, 4096
GROUP = 64
EPS = 1e-05
B = 2
N_CORES = 8
HPC = 4          # heads per core
OSL = 256        # out-feature slice per core
NF = 16          # tversky feature count
PVW = 17         # PV rhs width: 16 feat cols + ones col

BF = ml_dtypes.bfloat16


# ----------------------------------------------------------------- host math

def _ternary_deq(w: np.ndarray) -> np.ndarray:
    bf = ml_dtypes.bfloat16
    wb = w.astype(bf)
    wg = wb.reshape(-1, GROUP)
    m = (np.sum(np.abs(wg), axis=-1, keepdims=True, dtype=np.float32) / GROUP).astype(bf)
    scale = np.maximum(m.astype(np.float32), np.float32(1e-8)).astype(bf)
    ratio = (wg.astype(np.float32) / scale.astype(np.float32)).astype(bf)
    q = np.clip(np.round(ratio.astype(np.float32)), -1.0, 1.0).astype(bf)
    deq = (q.astype(np.float32) * scale.astype(np.float32)).astype(bf)
    return deq.reshape(wb.shape).astype(np.float32)


def _rope_tables(seqlen: int):
    rd = HD
    ar = np.arange(0, rd, 2, dtype=np.float32)
    inv_freq = 1.0 / ROPE_BASE ** (ar / rd)
    scale = TRAIN_LEN / YARN_MAX
    ramp = np.clip((ar / rd - 0.25) / 0.75, 0.0, 1.0)
    inv_freq = inv_freq / (ramp * (1.0 / scale - 1.0) + 1.0)
    freqs = np.arange(seqlen, dtype=np.float32)[:, None] * inv_freq[None, :]
    return np.cos(freqs).T.astype(np.float32), np.sin(freqs).T.astype(np.float32)


def _sigmoid(x):
    return 1.0 / (1.0 + np.exp(-x))


# ------------------------------------------------------------ device program

def build_program(S: int, dbg: bool = False):
    NC = S // 512            # 512-token chunks
    NT = S // 128            # 128-token k tiles

    nc = bacc.Bacc("TRN2", target_bir_lowering=False, debug=False,
                   num_devices=N_CORES)

    # DRAM I/O
    d_xT = nc.dram_tensor("xT", [DIM, S], BF16, kind="ExternalInput")
    # 640 = 256 q + 256 k + 128 vw-fold channels
    d_wT = nc.dram_tensor("wT", [DIM, 640], BF16, kind="ExternalInput")
    d_cosF = nc.dram_tensor("cosF", [128, S], BF16, kind="ExternalInput")
    d_sinF = nc.dram_tensor("sinF", [128, S], BF16, kind="ExternalInput")
    d_tri4 = nc.dram_tensor("tri4", [128, 512], BF16, kind="ExternalInput")
    d_mask8 = nc.dram_tensor("mask8", [128, 32], BF16, kind="ExternalInput")
    d_smask = nc.dram_tensor("smask", [128, 16], BF16, kind="ExternalInput")
    d_lngain = nc.dram_tensor("lngain", [8, 1], F32, kind="ExternalInput")
    d_ident = nc.dram_tensor("ident", [128, 128], BF16, kind="ExternalInput")
    d_AT = nc.dram_tensor("AT", [16, OSL], BF16, kind="ExternalInput")
    d_BT = nc.dram_tensor("BT", [16, OSL], BF16, kind="ExternalInput")
    d_out = nc.dram_tensor("out", [OSL, S], F32, kind="ExternalOutput")
    if dbg:
        d_dbg_xf = nc.dram_tensor("dbg_xf", [16, S], F32, kind="ExternalOutput")
        d_dbg_xfar = nc.dram_tensor("dbg_xfar", [16, S], F32, kind="ExternalOutput")

    with tile.TileContext(nc) as tc:
        persist = ExitStack()
        cpool = persist.enter_context(tc.tile_pool(name="consts", bufs=1))
        qkpool = persist.enter_context(tc.tile_pool(name="qk", bufs=1))
        vwpool = persist.enter_context(tc.tile_pool(name="vwrhs", bufs=1))
        xfpool = persist.enter_context(tc.tile_pool(name="xft", bufs=1))
        wpool = persist.enter_context(tc.tile_pool(name="wts", bufs=1))
        drpool = persist.enter_context(
            tc.tile_pool(name="drscratch", bufs=1, space="DRAM"))

        # transient pools
        xpool = persist.enter_context(tc.tile_pool(name="xstream", bufs=8))
        sqpool = persist.enter_context(tc.tile_pool(name="sq", bufs=4))
        scpool = persist.enter_context(tc.tile_pool(name="scb", bufs=4))
        s8pool = persist.enter_context(tc.tile_pool(name="sc8", bufs=2))
        rppool = persist.enter_context(tc.tile_pool(name="ropetmp", bufs=2))
        espool = persist.enter_context(tc.tile_pool(name="es", bufs=6))
        tailp = persist.enter_context(tc.tile_pool(name="tail", bufs=2))

        # PSUM budget (8 banks): proj 1 + aux 1 + waves 2x2 + pv 2 = 8
        proj_ps = persist.enter_context(
            tc.tile_pool(name="proj", bufs=1, space="PSUM"))
        aux_ps = persist.enter_context(
            tc.tile_pool(name="aux", bufs=1, space="PSUM"))
        wave_ps = persist.enter_context(
            tc.tile_pool(name="wave", bufs=2, space="PSUM"))
        pv_ps = persist.enter_context(
            tc.tile_pool(name="pv", bufs=1, space="PSUM"))

        def aux_tile():
            # single shared [128, 512] psum bank; callers slice what they need
            return aux_ps.tile([128, 512], F32, tag="aux", name="aux")

        # ---- persistent SBUF ----
        cosF = cpool.tile([128, S], BF16, name="cosF")
        sinF = cpool.tile([128, S], BF16, name="sinF")
        tri4 = cpool.tile([128, 512], BF16, name="tri4")
        mask8 = cpool.tile([128, 32], BF16, name="mask8")
        smask = cpool.tile([128, 16], BF16, name="smask")
        lngain = cpool.tile([8, 1], F32, name="lngain")
        eps8 = cpool.tile([8, 1], F32, name="eps8")
        nc.vector.memset(eps8[:], EPS)
        ident = cpool.tile([128, 128], BF16, name="ident")
        AT = cpool.tile([16, OSL], BF16, name="AT")
        BT = cpool.tile([16, OSL], BF16, name="BT")
        nc.sync.dma_start(cosF[:], d_cosF[:])
        nc.sync.dma_start(sinF[:], d_sinF[:])
        nc.sync.dma_start(tri4[:], d_tri4[:])
        nc.sync.dma_start(mask8[:], d_mask8[:])
        nc.sync.dma_start(smask[:], d_smask[:])
        nc.sync.dma_start(lngain[:], d_lngain[:])
        nc.sync.dma_start(ident[:], d_ident[:])
        nc.sync.dma_start(AT[:], d_AT[:])
        nc.sync.dma_start(BT[:], d_BT[:])

        # Pre-load the exp+ln table set so the placement pass never needs to
        # thrash between exp_and_others / natural_log per chunk.
        tables = list(__import__("concourse.hw_specs", fromlist=["x"])
                      .get_activation_tables(nc.m.arch).keys())
        set_id = tables.index("natural_log_exp_and_others")
        nc.scalar.add_instruction(mybir.InstLoadActFuncSet(
            name=nc.get_next_instruction_name(), act_func_set_id=set_id,
            ins=[], outs=[]))

        wts = [wpool.tile([128, 640], BF16, name=f"w{d}") for d in range(8)]
        for d in range(8):
            nc.sync.dma_start(wts[d][:], d_wT[d * 128:(d + 1) * 128, :])

        qa = [qkpool.tile([128, S], BF16, name=f"qa{t}") for t in range(2)]
        ka = [qkpool.tile([128, S], BF16, name=f"ka{t}") for t in range(2)]
        # PV rhs per ktile: [k 128, 8 problems x 17]; ones col at 16 mod 17
        rhs_vw = [vwpool.tile([128, 8 * PVW], BF16, name=f"rvw{t}")
                  for t in range(NT)]
        for t in range(NT):
            nc.vector.memset(
                rhs_vw[t][:].rearrange("p (g c) -> p g c", c=PVW)[:, :, 16:17],
                1.0)
        # xf^T strips per tile-group: rows 32p..32p+16 = (xf contrib | den)
        xft = [xfpool.tile([128, S], F32, name=f"xft{t}") for t in range(2)]

        # ---------------- emission helpers ----------------

        def proj_chunk_steps(c):
            """Returns a list of closures emitting projection of chunk c."""
            s0 = c * 512
            steps = []
            xt = [None] * 8
            sq_t = [None] * 4
            stat = [None]
            sc8 = [None]

            def load_x():
                for d in range(8):
                    xt[d] = xpool.tile([128, 512], BF16, tag="xt", name="xt")
                    nc.sync.dma_start(xt[d][:], d_xT[d * 128:(d + 1) * 128,
                                                     s0:s0 + 512])
            steps.append(load_x)

            # 4 qk chains: ot 0,1 = q tiles, ot 2,3 = k tiles
            def make_qk(ot):
                def f():
                    dst = qa[ot] if ot < 2 else ka[ot - 2]
                    pq = proj_ps.tile([128, 512], F32, tag="pmm", name="pmm")
                    for d in range(8):
                        nc.tensor.matmul(pq[:], wts[d][:, ot * 128:(ot + 1) * 128],
                                         xt[d][:], start=(d == 0), stop=(d == 7))
                    # unscaled drain (scale applied later in-place)
                    nc.vector.tensor_copy(dst[:, s0:s0 + 512], pq[:])
                    sq = sqpool.tile([128, 512], BF16, tag="sq", name="sq")
                    nc.vector.tensor_mul(sq[:], dst[:, s0:s0 + 512],
                                         dst[:, s0:s0 + 512])
                    sq_t[ot] = sq
                return f
            for ot in range(4):
                steps.append(make_qk(ot))

            def stats():
                st = aux_tile()
                for ot in range(4):
                    nc.tensor.matmul(st[0:8, :], mask8[:, ot * 8:(ot + 1) * 8],
                                     sq_t[ot][:], start=(ot == 0), stop=(ot == 3))
                lnt = s8pool.tile([8, 512], F32, tag="lnt", name="lnt")
                nc.scalar.activation(lnt[:], st[0:8, :], AF.Ln, scale=1.0 / HD,
                                     bias=eps8[:])
                s8 = s8pool.tile([8, 512], BF16, tag="sc8", name="sc8")
                nc.scalar.activation(s8[:], lnt[:], AF.Exp, scale=-0.5,
                                     bias=lngain[:])
                sc8[0] = s8
            steps.append(stats)

            def scale_rope():
                # broadcast row scales into block layout (via DRAM scratch --
                # SBUF sources cannot have a zero partition step), then
                # scale+rope
                dr8 = drpool.tile([8, 512], BF16, tag="dr8", bufs=2, name="dr8")
                nc.sync.dma_start(dr8[:], sc8[0][:])
                scb = []
                for tt in range(4):          # 2 q tiles then 2 k tiles
                    sb = scpool.tile([128, 512], BF16, tag="scb", name="scb")
                    for hh in range(2):
                        row = (tt // 2) * 4 + (tt % 2) * 2 + hh
                        nc.gpsimd.dma_start(
                            sb[hh * 64:hh * 64 + 64, :],
                            dr8[row:row + 1, :].to_broadcast([64, 512]))
                    scb.append(sb)
                tiles = [qa[0], qa[1], ka[0], ka[1]]
                for tt in range(4):
                    nc.vector.tensor_mul(tiles[tt][:, s0:s0 + 512],
                                         tiles[tt][:, s0:s0 + 512], scb[tt][:])
                # rope: prefetch partner blocks (adjacent 32-row block), then
                # x = x*cos + prt*sinF (sinF carries the half sign)
                for tt in range(4):
                    prt = rppool.tile([128, 512], BF16, tag="prt", name="prt")
                    for p in range(4):
                        nc.sync.dma_start(
                            prt[32 * p:32 * p + 32, :],
                            tiles[tt][32 * (p ^ 1):32 * (p ^ 1) + 32, s0:s0 + 512])
                    tb = rppool.tile([128, 512], BF16, tag="tb", name="tb")
                    nc.vector.tensor_mul(tb[:], prt[:], sinF[:, s0:s0 + 512])
                    nc.vector.tensor_mul(tiles[tt][:, s0:s0 + 512],
                                         tiles[tt][:, s0:s0 + 512],
                                         cosF[:, s0:s0 + 512])
                    nc.vector.tensor_add(tiles[tt][:, s0:s0 + 512],
                                         tiles[tt][:, s0:s0 + 512], tb[:])
            steps.append(scale_rope)

            def vw_chain():
                pv = proj_ps.tile([128, 512], F32, tag="pmm", name="pmm")
                for d in range(8):
                    nc.tensor.matmul(pv[:], wts[d][:, 512:640], xt[d][:],
                                     start=(d == 0), stop=(d == 7))
                vw_sb = sqpool.tile([128, 512], BF16, tag="vwsb", name="vwsb")
                nc.vector.tensor_copy(vw_sb[:], pv[:])
                # transpose each 128-token block: vw_sb [ch 128, s] -> [s, ch]
                for i in range(4):
                    pt = aux_tile()
                    nc.tensor.matmul(pt[:, 0:128], vw_sb[:, i * 128:(i + 1) * 128],
                                     ident[:], start=True, stop=True)
                    t = c * 4 + i
                    dst = rhs_vw[t][:].rearrange(
                        "p (g c) -> p g c", c=PVW)[:, :, 0:16]
                    nc.vector.tensor_copy(
                        dst, pt[:, 0:128].rearrange("p (g c) -> p g c", c=16))
            steps.append(vw_chain)
            return steps

        # attention state: per chunk j, accumulate xf strips in 2 psum banks
        def attn_chunk(j, inject):
            """Emit attention for q chunk j; call inject() between t-steps to
            interleave next chunk's projection work."""
            ntk = 4 * (j + 1)
            xfa = [pv_ps.tile([128, 512], F32, tag=f"xfa{qt}", name=f"xfa{qt}")
                   for qt in range(2)]
            es_q = {}
            LAG = 2

            def emit_pv(t):
                off = max(0, (t - 4 * j) * 128)
                for qt in range(2):
                    es = es_q.pop((t, qt))
                    es3 = es[:].rearrange("p (g c) -> p g c", c=512)
                    for p in range(4):
                        nc.tensor.matmul(
                            xfa[qt][32 * p:32 * p + PVW, off:512],
                            rhs_vw[t][:, (4 * qt + p) * PVW:(4 * qt + p + 1) * PVW],
                            es3[:, p, off:512],
                            start=(t == 0), stop=(t == ntk - 1),
                            tile_position=(0, 32 * p),
                            skip_group_check=True)

            for t in range(ntk):
                off = max(0, (t - 4 * j) * 128)
                w = 512 - off
                for qt in range(2):
                    es = espool.tile([128, 2048], BF16, tag="es", name="es")
                    for pair in range(2):
                        ps = wave_ps.tile([128, 1024], F32, tag="wv", name="wv")
                        for pp in range(2):
                            p = pair * 2 + pp
                            nc.tensor.matmul(
                                ps[:, pp * 512 + off:(pp + 1) * 512],
                                ka[qt][32 * p:32 * p + 32, t * 128:(t + 1) * 128],
                                qa[qt][32 * p:32 * p + 32,
                                       j * 512 + off:(j + 1) * 512],
                                start=True, stop=True,
                                tile_position=(32 * p, 0))
                        ps3 = ps[:].rearrange("p (g c) -> p g c", c=512)
                        es3 = es[:].rearrange("p (g c) -> p g c", c=512)
                        nc.scalar.activation(
                            es3[:, 2 * pair:2 * pair + 2, off:512],
                            ps3[:, :, off:512], AF.Exp)
                    if t >= 4 * j:
                        # causal mask on the diagonal 128-block of each strip
                        dv = es[:].rearrange("p (g c) -> p g c", c=512)[
                            :, :, off:off + 128]
                        tri = tri4[:].rearrange("p (g c) -> p g c", c=128)
                        nc.vector.tensor_mul(dv, dv, tri)
                    es_q[(t, qt)] = es
                if t >= LAG:
                    emit_pv(t - LAG)
                inject()
            for t in range(max(0, ntk - LAG), ntk):
                emit_pv(t)

            # drain strips to SBUF
            for qt in range(2):
                nc.vector.tensor_copy(xft[qt][:, j * 512:(j + 1) * 512],
                                      xfa[qt][:])

        # ---------------- tversky tail (per chunk) ----------------
        cc_in = [drpool.tile([16, 512], F32, name=f"ccin{j}") for j in range(NC)]
        cc_out = [drpool.tile([16, 512], F32, name=f"ccout{j}") for j in range(NC)]

        def tail_chunk_steps(j):
            s0 = j * 512
            st = {}

            def t1_dens():
                # gather dens rows (strip row 16 of each 32-block), then
                # reciprocal = exp(-ln) on ACT (set stays resident), stage
                # to DRAM for the partition-broadcast
                dens = tailp.tile([8, 512], F32, tag="dens", name="dens")
                for qt in range(2):
                    for p in range(4):
                        nc.gpsimd.dma_start(
                            dens[qt * 4 + p:qt * 4 + p + 1, :],
                            xft[qt][32 * p + 16:32 * p + 17, s0:s0 + 512])
                lnd = tailp.tile([8, 512], F32, tag="lnd", name="lnd")
                nc.scalar.activation(lnd[:], dens[:], AF.Ln)
                rb = tailp.tile([8, 512], F32, tag="rb", name="rb")
                nc.scalar.activation(rb[:], lnd[:], AF.Exp, scale=-1.0)
                drb = drpool.tile([8, 512], F32, tag="drb", bufs=2, name="drb")
                nc.sync.dma_start(drb[:], rb[:])
                st["drb"] = drb

            def t2_xf():
                drb = st["drb"]
                rbb = [tailp.tile([128, 512], F32, tag=f"rbb{qt}", name="rbb")
                       for qt in range(2)]
                for qt in range(2):
                    for p in range(4):
                        nc.gpsimd.dma_start(
                            rbb[qt][32 * p:32 * p + 16, :],
                            drb[qt * 4 + p:qt * 4 + p + 1, :]
                            .to_broadcast([16, 512]))
                sc = [tailp.tile([128, 512], BF16, tag=f"sc{qt}", name="sc")
                      for qt in range(2)]
                for qt in range(2):
                    nc.vector.tensor_mul(sc[qt][:],
                                         xft[qt][:, s0:s0 + 512], rbb[qt][:])
                pxf = aux_tile()
                for qt in range(2):
                    nc.tensor.matmul(pxf[0:16, :], smask[:], sc[qt][:],
                                     start=(qt == 0), stop=(qt == 1))
                xfl = tailp.tile([16, 512], F32, tag="xfl", name="xfl")
                nc.vector.tensor_copy(xfl[:], pxf[0:16, :])
                if dbg:
                    nc.sync.dma_start(d_dbg_xf[:, s0:s0 + 512], xfl[:])
                nc.sync.dma_start(cc_in[j][:], xfl[:])
                nc.gpsimd.collective_compute(
                    "AllReduce", ALU.add,
                    replica_groups=[[0, 1, 2, 3], [4, 5, 6, 7]],
                    ins=[cc_in[j][:]], outs=[cc_out[j][:]])

            def t3_out():
                xf = tailp.tile([16, 512], F32, tag="xfr", name="xfr")
                nc.sync.dma_start(xf[:], cc_out[j][:])
                if dbg:
                    nc.sync.dma_start(d_dbg_xfar[:, s0:s0 + 512], xf[:])
                # xa = xf*sig(5xf) = xf/(1+e), oms = e/(1+e), e = exp(-5 xf)
                e = tailp.tile([16, 512], F32, tag="e", name="e")
                nc.scalar.activation(e[:], xf[:], AF.Exp, scale=-5.0)
                t1 = tailp.tile([16, 512], F32, tag="t1", name="t1")
                nc.vector.tensor_scalar_add(t1[:], e[:], 1.0)
                lt = tailp.tile([16, 512], F32, tag="lt", name="lt")
                nc.scalar.activation(lt[:], t1[:], AF.Ln)
                r = tailp.tile([16, 512], F32, tag="r", name="r")
                nc.scalar.activation(r[:], lt[:], AF.Exp, scale=-1.0)
                xa = tailp.tile([16, 512], BF16, tag="xa", name="xa")
                oms = tailp.tile([16, 512], BF16, tag="oms", name="oms")
                nc.vector.tensor_mul(xa[:], xf[:], r[:])
                nc.vector.tensor_mul(oms[:], e[:], r[:])
                for ot in range(2):
                    po = aux_tile()
                    nc.tensor.matmul(po[:], AT[:, ot * 128:(ot + 1) * 128],
                                     xa[:], start=True, stop=False)
                    nc.tensor.matmul(po[:], BT[:, ot * 128:(ot + 1) * 128],
                                     oms[:], start=False, stop=True)
                    ob = tailp.tile([128, 512], F32, tag="ob", name="ob")
                    nc.vector.tensor_copy(ob[:], po[:])
                    nc.sync.dma_start(
                        d_out[ot * 128:(ot + 1) * 128, s0:s0 + 512], ob[:])

            return [t1_dens, t2_xf, t3_out]

        # ---------------- main schedule ----------------
        # attn(j) runs with proj(j+1) and tail(j-1) steps injected between
        # t-iterations so the PE never sits behind a long-latency tail chain.
        for step in proj_chunk_steps(0):
            step()
        for j in range(NC):
            pending = []
            if j >= 1:
                pending += tail_chunk_steps(j - 1)
            if j + 1 < NC:
                # interleave proj steps after the tail kickoff steps
                pending += proj_chunk_steps(j + 1)
            counter = [0]
            total_t = 4 * (j + 1)
            nsteps = len(pending)

            def inject():
                counter[0] += 1
                want = (counter[0] * nsteps) // total_t
                while len(pending) and (nsteps - len(pending)) < want:
                    pending.pop(0)()
            attn_chunk(j, inject)
            while pending:
                pending.pop(0)()
        for step in tail_chunk_steps(NC - 1):
            step()

        persist.close()

    nc.compile()
    return nc


# ----------------------------------------------------------- host marshaling

def make_in_maps(S, x, w_qkv, features, prototypes, theta, alpha, beta,
                 q_gain, diff_lambda):
    x = np.asarray(x, np.float32)
    w_qkv = np.asarray(w_qkv, np.float32)
    features = np.asarray(features, np.float32)
    prototypes = np.asarray(prototypes, np.float32)
    theta = float(np.abs(np.asarray(theta, np.float32)))
    alpha = float(np.abs(np.asarray(alpha, np.float32)))
    beta = float(np.abs(np.asarray(beta, np.float32)))
    q_gain = np.asarray(q_gain, np.float32)
    lam = np.asarray(diff_lambda, np.float32)

    w_deq = _ternary_deq(w_qkv)
    p_deq = _ternary_deq(prototypes)
    cosT, sinT = _rope_tables(S)       # [32, S]

    rows = np.arange(128)
    sgn = np.where((rows // 32) % 2 == 0, 1.0, -1.0).astype(np.float32)
    cosF = cosT[rows % 32, :].astype(BF)
    sinF = (sinT[rows % 32, :] * sgn[:, None]).astype(BF)

    # diag-block causal mask, tiled 4x horizontally: [128, 512]
    tri = (np.arange(128)[None, :] >= np.arange(128)[:, None]).astype(np.float32)
    tri4 = np.tile(tri, (1, 4)).astype(BF)

    # stats masks: [128, 32] = 4 tile-types x 8 stat rows
    mask8 = np.zeros((128, 32), np.float32)
    for tt in range(4):
        for r in range(128):
            head_in_tile = r // 64
            row = (tt // 2) * 4 + (tt % 2) * 2 + head_in_tile
            mask8[r, tt * 8 + row] = 1.0
    mask8 = mask8.astype(BF)

    # strip-sum mask: rows 32p+i (i<16) -> col i
    smask = np.zeros((128, 16), np.float32)
    for p in range(4):
        for i in range(16):
            smask[32 * p + i, i] = 1.0

    ident = np.eye(128, dtype=np.float32).astype(BF)

    in_maps = []
    for c in range(N_CORES):
        b, hg = c // 4, c % 4
        h0 = hg * HPC
        qrows = slice(h0 * HD, (h0 + HPC) * HD)
        o0 = hg * OSL

        wq = w_deq[0:DIM][qrows]                   # [256, 1024]
        wk = w_deq[DIM:2 * DIM][qrows]             # [256, 1024]
        wv = w_deq[2 * DIM:3 * DIM][qrows]         # [256, 1024]

        # vw fold: per (h, f) channel block [k,16] = x @ (wv_f^T W_hf^T)
        # W_h0 = M1 + M2, W_h1 = lam_h (M2 - M1), M = features[:, head dims]
        wfold = np.zeros((DIM, 8 * NF), np.float32)
        for h in range(HPC):
            gh = h0 + h
            M1 = features[:, gh * 64:gh * 64 + 32]       # [16, 32]
            M2 = features[:, gh * 64 + 32:gh * 64 + 64]
            Wh0 = (M1 + M2)                               # [16, 32]
            Wh1 = lam[gh] * (M2 - M1)
            v0 = wv[h * 64:h * 64 + 32]                   # [32, 1024]
            v1 = wv[h * 64 + 32:h * 64 + 64]
            wfold[:, (2 * h) * NF:(2 * h + 1) * NF] = v0.T @ Wh0.T
            wfold[:, (2 * h + 1) * NF:(2 * h + 2) * NF] = v1.T @ Wh1.T

        wT = np.concatenate([wq.T, wk.T, wfold], axis=1)  # [1024, 640]

        gains = q_gain[h0:h0 + HPC] / math.sqrt(HD // 2)
        assert np.all(gains > 0), "nonpositive q_gain unsupported by ln-fold"
        lngain = np.zeros((8, 1), np.float32)
        lngain[0:4, 0] = np.log(gains)

        p_f = p_deq[o0:o0 + OSL] @ features.T          # [256, 16]
        p_s = _sigmoid(5.0 * p_f)
        p_a = p_f * p_s
        A_eff = theta * p_a - alpha * (1.0 - p_s)
        B_eff = -beta * p_a

        m = {
            "xT": np.ascontiguousarray(x[b].T).astype(BF),
            "wT": np.ascontiguousarray(wT).astype(BF),
            "cosF": cosF, "sinF": sinF,
            "tri4": tri4, "mask8": mask8,
            "smask": smask.astype(BF),
            "lngain": lngain,
            "ident": ident,
            "AT": np.ascontiguousarray(A_eff.T).astype(BF),
            "BT": np.ascontiguousarray(B_eff.T).astype(BF),
        }
        in_maps.append(m)
    return in_maps


def assemble_output(S, results):
    out = np.empty((B, S, DIM), np.float32)
    for c in range(N_CORES):
        b, hg = c // 4, c % 4
        out[b, :, hg * OSL:(hg + 1) * OSL] = results[c]["out"].T
    return out


_PROGRAM_CACHE = {}


def kernel(x, w_qkv, features, prototypes, theta, alpha, beta, q_gain,
           diff_lambda, _trace=False):
    x = np.asarray(x, np.float32)
    S = x.shape[1]
    if S not in _PROGRAM_CACHE:
        _PROGRAM_CACHE[S] = build_program(S)
    nc = _PROGRAM_CACHE[S]

    in_maps = make_in_maps(S, x, w_qkv, features, prototypes, theta, alpha,
                           beta, q_gain, diff_lambda)
    res = run_bass_kernel_spmd(nc, in_maps, list(range(N_CORES)),
                               trace=_trace)
    out = assemble_output(S, res.results)
    if _trace:
        return out, res
    return out


# revision 29
# speedup vs baseline: 1.9670x; 1.0299x over previous
"""Trainium2 Bass kernel for nn_CausalSelfAttention_68496138437292.

Sharding: 8 cores = 2 batches x 4 head-groups; core c handles batch c//4 and
local heads [4*(c%4), 4*(c%4)+4).  The Tversky projection is sharded over
out_features (each core computes a 256-wide o-slice); the 16-wide feature
contraction x_f (summed over D, split across head-groups) uses one small
AllReduce per 512-token chunk over each batch's 4-core group.

Key structural ideas (vs the phase-serial f32r baseline):
- bf16 everywhere on the PE; all (head, half) channel blocks are packed 4 per
  128-partition tile in natural order, so projection PSUM drains straight into
  the attention layout with no DMA repacking.
- The attention OUTPUT is never materialized: the final output only needs
  x_f = feat . y^T, and  feat_slice . (P V / den)^T = (P (V W^T))^T / den,
  with  V W^T = x @ (w_v^T W^T)  folded into the projection as 128 extra
  channels (host-precomputed fold).  Attention per (head, half) reduces to
  scores -> exp -> one 17-wide PV matmul (16 feat cols + a ones column that
  emits the softmax denominator for free).
- Scores/PV use PE array tiling (tile_position) for ~1.4x matmul throughput;
  exp runs as [128, 1024] ACT ops over multi-bank PSUM with trimmed causal
  widths; rmsnorm rsqrt+gain is one ln + one exp (single ACT table set).
- Projection of chunk c+1 is interleaved into attention of chunk j=c so the
  PE keeps busy while ACT chews exp; Tversky tail is pipelined per chunk with
  one [16, 512] AllReduce each.
"""

import math
from contextlib import ExitStack

import ml_dtypes
import numpy as np

import concourse.bass as bass
import concourse.mybir as mybir
import concourse.tile as tile
from concourse import bacc
from concourse.bass_utils import run_bass_kernel_spmd

F32 = mybir.dt.float32
F32R = mybir.dt.float32r
BF16 = mybir.dt.bfloat16
AF = mybir.ActivationFunctionType
ALU = mybir.AluOpType

DIM, NH, HD = 1024, 16, 64
ROPE_BASE, TRAIN_LEN, YARN_MAX = 10000.0, 1024, 4096
GROUP = 64
EPS = 1e-05
B = 2
N_CORES = 8
HPC = 4          # heads per core
OSL = 256        # out-feature slice per core
NF = 16          # tversky feature count
PVW = 17         # PV rhs width: 16 feat cols + ones col

BF = ml_dtypes.bfloat16


# ----------------------------------------------------------------- host math

def _ternary_deq(w: np.ndarray) -> np.ndarray:
    bf = ml_dtypes.bfloat16
    wb = w.astype(bf)
    wg = wb.reshape(-1, GROUP)
    m = (np.sum(np.abs(wg), axis=-1, keepdims=True, dtype=np.float32) / GROUP).astype(bf)
    scale = np.maximum(m.astype(np.float32), np.float32(1e-8)).astype(bf)
    ratio = (wg.astype(np.float32) / scale.astype(np.float32)).astype(bf)
    q = np.clip(np.round(ratio.astype(np.float32)), -1.0, 1.0).astype(bf)
    deq = (q.astype(np.float32) * scale.astype(np.float32)).astype(bf)
    return deq.reshape(wb.shape).astype(np.float32)


def _rope_tables(seqlen: int):
    rd = HD
    ar = np.arange(0, rd, 2, dtype=np.float32)
    inv_freq = 1.0 / ROPE_BASE ** (ar / rd)
    scale = TRAIN_LEN / YARN_MAX
    ramp = np.clip((ar / rd - 0.25) / 0.75, 0.0, 1.0)
    inv_freq = inv_freq / (ramp * (1.0 / scale - 1.0) + 1.0)
    freqs = np.arange(seqlen, dtype=np.float32)[:, None] * inv_freq[None, :]
    return np.cos(freqs).T.astype(np.float32), np.sin(freqs).T.astype(np.float32)


def _sigmoid(x):
    return 1.0 / (1.0 + np.exp(-x))


# ------------------------------------------------------------ device program

def build_program(S: int, dbg: bool = False):
    NC = S // 512            # 512-token chunks
    NT = S // 128            # 128-token k tiles

    nc = bacc.Bacc("TRN2", target_bir_lowering=False, debug=False,
                   num_devices=N_CORES)

    # DRAM I/O
    d_xT = nc.dram_tensor("xT", [DIM, S], BF16, kind="ExternalInput")
    # 640 = 256 q + 256 k + 128 vw-fold channels
    d_wT = nc.dram_tensor("wT", [DIM, 640], BF16, kind="ExternalInput")
    d_cosF = nc.dram_tensor("cosF", [128, S], BF16, kind="ExternalInput")
    d_sinF = nc.dram_tensor("sinF", [128, S], BF16, kind="ExternalInput")
    d_tri4 = nc.dram_tensor("tri4", [128, 512], BF16, kind="ExternalInput")
    d_mask8 = nc.dram_tensor("mask8", [128, 32], BF16, kind="ExternalInput")
    d_smask = nc.dram_tensor("smask", [128, 16], BF16, kind="ExternalInput")
    d_lngain = nc.dram_tensor("lngain", [8, 1], F32, kind="ExternalInput")
    d_ident = nc.dram_tensor("ident", [128, 128], BF16, kind="ExternalInput")
    d_AT = nc.dram_tensor("AT", [16, OSL], BF16, kind="ExternalInput")
    d_BT = nc.dram_tensor("BT", [16, OSL], BF16, kind="ExternalInput")
    d_out = nc.dram_tensor("out", [OSL, S], F32, kind="ExternalOutput")
    if dbg:
        d_dbg_xf = nc.dram_tensor("dbg_xf", [16, S], F32, kind="ExternalOutput")
        d_dbg_xfar = nc.dram_tensor("dbg_xfar", [16, S], F32, kind="ExternalOutput")

    with tile.TileContext(nc) as tc:
        persist = ExitStack()
        cpool = persist.enter_context(tc.tile_pool(name="consts", bufs=1))
        qkpool = persist.enter_context(tc.tile_pool(name="qk", bufs=1))
        vwpool = persist.enter_context(tc.tile_pool(name="vwrhs", bufs=1))
        xfpool = persist.enter_context(tc.tile_pool(name="xft", bufs=1))
        wpool = persist.enter_context(tc.tile_pool(name="wts", bufs=1))
        drpool = persist.enter_context(
            tc.tile_pool(name="drscratch", bufs=1, space="DRAM"))

        # transient pools
        xpool = persist.enter_context(tc.tile_pool(name="xstream", bufs=8))
        sqpool = persist.enter_context(tc.tile_pool(name="sq", bufs=4))
        scpool = persist.enter_context(tc.tile_pool(name="scb", bufs=4))
        s8pool = persist.enter_context(tc.tile_pool(name="sc8", bufs=2))
        rppool = persist.enter_context(tc.tile_pool(name="ropetmp", bufs=2))
        espool = persist.enter_context(tc.tile_pool(name="es", bufs=6))
        tailp = persist.enter_context(tc.tile_pool(name="tail", bufs=2))

        # PSUM budget (8 banks): proj 1 + aux 1 + waves 2x2 + pv 2 = 8
        proj_ps = persist.enter_context(
            tc.tile_pool(name="proj", bufs=1, space="PSUM"))
        aux_ps = persist.enter_context(
            tc.tile_pool(name="aux", bufs=1, space="PSUM"))
        wave_ps = persist.enter_context(
            tc.tile_pool(name="wave", bufs=2, space="PSUM"))
        pv_ps = persist.enter_context(
            tc.tile_pool(name="pv", bufs=1, space="PSUM"))

        def aux_tile():
            # single shared [128, 512] psum bank; callers slice what they need
            return aux_ps.tile([128, 512], F32, tag="aux", name="aux")

        # ---- persistent SBUF ----
        # weights first: the first projection chain only needs wts + x(0)
        wts0 = wpool.tile([128, 640], BF16, name="w0")
        nc.sync.dma_start(wts0[:], d_wT[0:128, :])
        cosF = cpool.tile([128, S], BF16, name="cosF")
        sinF = cpool.tile([128, S], BF16, name="sinF")
        tri4 = cpool.tile([128, 512], BF16, name="tri4")
        mask8 = cpool.tile([128, 32], BF16, name="mask8")
        smask = cpool.tile([128, 16], BF16, name="smask")
        lngain = cpool.tile([8, 1], F32, name="lngain")
        eps8 = cpool.tile([8, 1], F32, name="eps8")
        nc.vector.memset(eps8[:], EPS)
        ident = cpool.tile([128, 128], BF16, name="ident")
        AT = cpool.tile([16, OSL], BF16, name="AT")
        BT = cpool.tile([16, OSL], BF16, name="BT")

        # Pre-load the exp+ln table set so the placement pass never needs to
        # thrash between exp_and_others / natural_log per chunk.
        tables = list(__import__("concourse.hw_specs", fromlist=["x"])
                      .get_activation_tables(nc.m.arch).keys())
        set_id = tables.index("natural_log_exp_and_others")
        nc.scalar.add_instruction(mybir.InstLoadActFuncSet(
            name=nc.get_next_instruction_name(), act_func_set_id=set_id,
            ins=[], outs=[]))

        wts = [wts0] + [wpool.tile([128, 640], BF16, name=f"w{d}")
                        for d in range(1, 8)]
        for d in range(1, 8):
            nc.sync.dma_start(wts[d][:], d_wT[d * 128:(d + 1) * 128, :])
        # consts go on the gpsimd DMA queue so they don't delay weights/x
        nc.gpsimd.dma_start(cosF[:], d_cosF[:])
        nc.gpsimd.dma_start(sinF[:], d_sinF[:])
        nc.gpsimd.dma_start(tri4[:], d_tri4[:])
        nc.gpsimd.dma_start(mask8[:], d_mask8[:])
        nc.gpsimd.dma_start(smask[:], d_smask[:])
        nc.gpsimd.dma_start(lngain[:], d_lngain[:])
        nc.gpsimd.dma_start(ident[:], d_ident[:])
        nc.gpsimd.dma_start(AT[:], d_AT[:])
        nc.gpsimd.dma_start(BT[:], d_BT[:])

        qa = [qkpool.tile([128, S], BF16, name=f"qa{t}") for t in range(2)]
        ka = [qkpool.tile([128, S], BF16, name=f"ka{t}") for t in range(2)]
        # PV rhs per ktile: [k 128, 8 problems x 17]; ones col at 16 mod 17
        rhs_vw = [vwpool.tile([128, 8 * PVW], BF16, name=f"rvw{t}")
                  for t in range(NT)]
        for t in range(NT):
            nc.vector.memset(
                rhs_vw[t][:].rearrange("p (g c) -> p g c", c=PVW)[:, :, 16:17],
                1.0)
        # xf^T strips per tile-group: rows 32p..32p+16 = (xf contrib | den)
        xft = [xfpool.tile([128, S], F32, name=f"xft{t}") for t in range(2)]

        # ---------------- emission helpers ----------------

        def proj_chunk_steps(c):
            """Returns a list of closures emitting projection of chunk c."""
            s0 = c * 512
            steps = []
            xt = [None] * 8
            sq_t = [None] * 4
            stat = [None]
            sc8 = [None]

            def load_x():
                for d in range(8):
                    xt[d] = xpool.tile([128, 512], BF16, tag="xt", name="xt")
                    nc.sync.dma_start(xt[d][:], d_xT[d * 128:(d + 1) * 128,
                                                     s0:s0 + 512])
            steps.append(load_x)

            # 4 qk chains: ot 0,1 = q tiles, ot 2,3 = k tiles
            def make_qk(ot):
                def f():
                    dst = qa[ot] if ot < 2 else ka[ot - 2]
                    pq = proj_ps.tile([128, 512], F32, tag="pmm", name="pmm")
                    for d in range(8):
                        nc.tensor.matmul(pq[:], wts[d][:, ot * 128:(ot + 1) * 128],
                                         xt[d][:], start=(d == 0), stop=(d == 7))
                    # unscaled drain (scale applied later in-place)
                    nc.vector.tensor_copy(dst[:, s0:s0 + 512], pq[:])
                    sq = sqpool.tile([128, 512], BF16, tag="sq", name="sq")
                    nc.vector.tensor_mul(sq[:], dst[:, s0:s0 + 512],
                                         dst[:, s0:s0 + 512])
                    sq_t[ot] = sq
                return f
            for ot in range(4):
                steps.append(make_qk(ot))

            def stats():
                st = aux_tile()
                for ot in range(4):
                    nc.tensor.matmul(st[0:8, :], mask8[:, ot * 8:(ot + 1) * 8],
                                     sq_t[ot][:], start=(ot == 0), stop=(ot == 3))
                lnt = s8pool.tile([8, 512], F32, tag="lnt", name="lnt")
                nc.scalar.activation(lnt[:], st[0:8, :], AF.Ln, scale=1.0 / HD,
                                     bias=eps8[:])
                s8 = s8pool.tile([8, 512], BF16, tag="sc8", name="sc8")
                nc.scalar.activation(s8[:], lnt[:], AF.Exp, scale=-0.5,
                                     bias=lngain[:])
                sc8[0] = s8
            steps.append(stats)

            def scale_rope():
                # broadcast row scales into block layout (via DRAM scratch --
                # SBUF sources cannot have a zero partition step), then
                # scale+rope
                dr8 = drpool.tile([8, 512], BF16, tag="dr8", bufs=2, name="dr8")
                nc.sync.dma_start(dr8[:], sc8[0][:])
                scb = []
                for tt in range(4):          # 2 q tiles then 2 k tiles
                    sb = scpool.tile([128, 512], BF16, tag="scb", name="scb")
                    for hh in range(2):
                        row = (tt // 2) * 4 + (tt % 2) * 2 + hh
                        nc.gpsimd.dma_start(
                            sb[hh * 64:hh * 64 + 64, :],
                            dr8[row:row + 1, :].to_broadcast([64, 512]))
                    scb.append(sb)
                tiles = [qa[0], qa[1], ka[0], ka[1]]
                for tt in range(4):
                    nc.vector.tensor_mul(tiles[tt][:, s0:s0 + 512],
                                         tiles[tt][:, s0:s0 + 512], scb[tt][:])
                # rope: prefetch partner blocks (adjacent 32-row block), then
                # x = x*cos + prt*sinF (sinF carries the half sign)
                for tt in range(4):
                    prt = rppool.tile([128, 512], BF16, tag="prt", name="prt")
                    for p in range(4):
                        nc.sync.dma_start(
                            prt[32 * p:32 * p + 32, :],
                            tiles[tt][32 * (p ^ 1):32 * (p ^ 1) + 32, s0:s0 + 512])
                    tb = rppool.tile([128, 512], BF16, tag="tb", name="tb")
                    nc.vector.tensor_mul(tb[:], prt[:], sinF[:, s0:s0 + 512])
                    nc.vector.tensor_mul(tiles[tt][:, s0:s0 + 512],
                                         tiles[tt][:, s0:s0 + 512],
                                         cosF[:, s0:s0 + 512])
                    nc.vector.tensor_add(tiles[tt][:, s0:s0 + 512],
                                         tiles[tt][:, s0:s0 + 512], tb[:])
            steps.append(scale_rope)

            def vw_chain():
                pv = proj_ps.tile([128, 512], F32, tag="pmm", name="pmm")
                for d in range(8):
                    nc.tensor.matmul(pv[:], wts[d][:, 512:640], xt[d][:],
                                     start=(d == 0), stop=(d == 7))
                vw_sb = sqpool.tile([128, 512], BF16, tag="vwsb", name="vwsb")
                nc.vector.tensor_copy(vw_sb[:], pv[:])
                # transpose each 128-token block: vw_sb [ch 128, s] -> [s, ch]
                for i in range(4):
                    pt = aux_tile()
                    nc.tensor.matmul(pt[:, 0:128], vw_sb[:, i * 128:(i + 1) * 128],
                                     ident[:], start=True, stop=True)
                    t = c * 4 + i
                    dst = rhs_vw[t][:].rearrange(
                        "p (g c) -> p g c", c=PVW)[:, :, 0:16]
                    nc.vector.tensor_copy(
                        dst, pt[:, 0:128].rearrange("p (g c) -> p g c", c=16))
            steps.append(vw_chain)
            return steps

        # attention state: per chunk j, accumulate xf strips in 2 psum banks
        def attn_chunk(j, inject):
            """Emit attention for q chunk j; call inject() between t-steps to
            interleave next chunk's projection work."""
            ntk = 4 * (j + 1)
            xfa = [pv_ps.tile([128, 512], F32, tag=f"xfa{qt}", name=f"xfa{qt}")
                   for qt in range(2)]
            es_q = {}
            LAG = 2

            def emit_pv(t):
                off = max(0, (t - 4 * j) * 128)
                for qt in range(2):
                    es = es_q.pop((t, qt))
                    es3 = es[:].rearrange("p (g c) -> p g c", c=512)
                    for p in range(4):
                        nc.tensor.matmul(
                            xfa[qt][32 * p:32 * p + PVW, off:512],
                            rhs_vw[t][:, (4 * qt + p) * PVW:(4 * qt + p + 1) * PVW],
                            es3[:, p, off:512],
                            start=(t == 0), stop=(t == ntk - 1),
                            tile_position=(0, 32 * p),
                            skip_group_check=True)

            for t in range(ntk):
                off = max(0, (t - 4 * j) * 128)
                w = 512 - off
                for qt in range(2):
                    es = espool.tile([128, 2048], BF16, tag="es", name="es")
                    for pair in range(2):
                        ps = wave_ps.tile([128, 1024], F32, tag="wv", name="wv")
                        for pp in range(2):
                            p = pair * 2 + pp
                            nc.tensor.matmul(
                                ps[:, pp * 512 + off:(pp + 1) * 512],
                                ka[qt][32 * p:32 * p + 32, t * 128:(t + 1) * 128],
                                qa[qt][32 * p:32 * p + 32,
                                       j * 512 + off:(j + 1) * 512],
                                start=True, stop=True,
                                tile_position=(32 * p, 0))
                        ps3 = ps[:].rearrange("p (g c) -> p g c", c=512)
                        es3 = es[:].rearrange("p (g c) -> p g c", c=512)
                        nc.scalar.activation(
                            es3[:, 2 * pair:2 * pair + 2, off:512],
                            ps3[:, :, off:512], AF.Exp)
                    if t >= 4 * j:
                        # causal mask on the diagonal 128-block of each strip
                        dv = es[:].rearrange("p (g c) -> p g c", c=512)[
                            :, :, off:off + 128]
                        tri = tri4[:].rearrange("p (g c) -> p g c", c=128)
                        nc.vector.tensor_mul(dv, dv, tri)
                    es_q[(t, qt)] = es
                if t >= LAG:
                    emit_pv(t - LAG)
                inject()
            for t in range(max(0, ntk - LAG), ntk):
                emit_pv(t)

            # drain strips to SBUF
            for qt in range(2):
                nc.vector.tensor_copy(xft[qt][:, j * 512:(j + 1) * 512],
                                      xfa[qt][:])

        # ---------------- tversky tail (per chunk) ----------------
        cc_in = [drpool.tile([16, 512], F32, name=f"ccin{j}") for j in range(NC)]
        cc_out = [drpool.tile([16, 512], F32, name=f"ccout{j}") for j in range(NC)]

        def tail_chunk_steps(j):
            s0 = j * 512
            st = {}

            def t1_dens():
                # gather dens rows (strip row 16 of each 32-block), then
                # reciprocal = exp(-ln) on ACT (set stays resident), stage
                # to DRAM for the partition-broadcast
                dens = tailp.tile([8, 512], F32, tag="dens", name="dens")
                for qt in range(2):
                    for p in range(4):
                        nc.gpsimd.dma_start(
                            dens[qt * 4 + p:qt * 4 + p + 1, :],
                            xft[qt][32 * p + 16:32 * p + 17, s0:s0 + 512])
                lnd = tailp.tile([8, 512], F32, tag="lnd", name="lnd")
                nc.scalar.activation(lnd[:], dens[:], AF.Ln)
                rb = tailp.tile([8, 512], F32, tag="rb", name="rb")
                nc.scalar.activation(rb[:], lnd[:], AF.Exp, scale=-1.0)
                drb = drpool.tile([8, 512], F32, tag="drb", bufs=2, name="drb")
                nc.sync.dma_start(drb[:], rb[:])
                st["drb"] = drb

            def t2_xf():
                drb = st["drb"]
                rbb = [tailp.tile([128, 512], F32, tag=f"rbb{qt}", name="rbb")
                       for qt in range(2)]
                for qt in range(2):
                    for p in range(4):
                        nc.gpsimd.dma_start(
                            rbb[qt][32 * p:32 * p + 16, :],
                            drb[qt * 4 + p:qt * 4 + p + 1, :]
                            .to_broadcast([16, 512]))
                sc = [tailp.tile([128, 512], BF16, tag=f"sc{qt}", name="sc")
                      for qt in range(2)]
                for qt in range(2):
                    nc.vector.tensor_mul(sc[qt][:],
                                         xft[qt][:, s0:s0 + 512], rbb[qt][:])
                pxf = aux_tile()
                for qt in range(2):
                    nc.tensor.matmul(pxf[0:16, :], smask[:], sc[qt][:],
                                     start=(qt == 0), stop=(qt == 1))
                xfl = tailp.tile([16, 512], F32, tag="xfl", name="xfl")
                nc.vector.tensor_copy(xfl[:], pxf[0:16, :])
                if dbg:
                    nc.sync.dma_start(d_dbg_xf[:, s0:s0 + 512], xfl[:])
                nc.sync.dma_start(cc_in[j][:], xfl[:])
                nc.gpsimd.collective_compute(
                    "AllReduce", ALU.add,
                    replica_groups=[[0, 1, 2, 3], [4, 5, 6, 7]],
                    ins=[cc_in[j][:]], outs=[cc_out[j][:]])

            def t3_out():
                xf = tailp.tile([16, 512], F32, tag="xfr", name="xfr")
                nc.sync.dma_start(xf[:], cc_out[j][:])
                if dbg:
                    nc.sync.dma_start(d_dbg_xfar[:, s0:s0 + 512], xf[:])
                # xa = xf*sig(5xf) = xf/(1+e), oms = e/(1+e), e = exp(-5 xf)
                e = tailp.tile([16, 512], F32, tag="e", name="e")
                nc.scalar.activation(e[:], xf[:], AF.Exp, scale=-5.0)
                t1 = tailp.tile([16, 512], F32, tag="t1", name="t1")
                nc.vector.tensor_scalar_add(t1[:], e[:], 1.0)
                lt = tailp.tile([16, 512], F32, tag="lt", name="lt")
                nc.scalar.activation(lt[:], t1[:], AF.Ln)
                r = tailp.tile([16, 512], F32, tag="r", name="r")
                nc.scalar.activation(r[:], lt[:], AF.Exp, scale=-1.0)
                xa = tailp.tile([16, 512], BF16, tag="xa", name="xa")
                oms = tailp.tile([16, 512], BF16, tag="oms", name="oms")
                nc.vector.tensor_mul(xa[:], xf[:], r[:])
                nc.vector.tensor_mul(oms[:], e[:], r[:])
                for ot in range(2):
                    po = aux_tile()
                    nc.tensor.matmul(po[:], AT[:, ot * 128:(ot + 1) * 128],
                                     xa[:], start=True, stop=False)
                    nc.tensor.matmul(po[:], BT[:, ot * 128:(ot + 1) * 128],
                                     oms[:], start=False, stop=True)
                    ob = tailp.tile([128, 512], F32, tag="ob", name="ob")
                    nc.vector.tensor_copy(ob[:], po[:])
                    nc.sync.dma_start(
                        d_out[ot * 128:(ot + 1) * 128, s0:s0 + 512], ob[:])

            return [t1_dens, t2_xf, t3_out]

        # ---------------- main schedule ----------------
        # attn(j) runs with proj(j+1) and tail(j-1) steps injected between
        # t-iterations so the PE never sits behind a long-latency tail chain.
        for step in proj_chunk_steps(0):
            step()
        for j in range(NC):
            pending = []
            if j >= 1:
                pending += tail_chunk_steps(j - 1)
            if j + 1 < NC:
                # interleave proj steps after the tail kickoff steps
                pending += proj_chunk_steps(j + 1)
            counter = [0]
            # front-load: finish all injected steps by ~45% of the t loop so
            # the next chunk's q/k are roped before this chunk's waves end
            horizon = max(1, int(4 * (j + 1) * 0.45))
            nsteps = len(pending)

            def inject():
                counter[0] += 1
                want = min(nsteps, -(-counter[0] * nsteps // horizon))
                while len(pending) and (nsteps - len(pending)) < want:
                    pending.pop(0)()
            attn_chunk(j, inject)
            while pending:
                pending.pop(0)()
        for step in tail_chunk_steps(NC - 1):
            step()

        persist.close()

    nc.compile()
    return nc


# ----------------------------------------------------------- host marshaling

def make_in_maps(S, x, w_qkv, features, prototypes, theta, alpha, beta,
                 q_gain, diff_lambda):
    x = np.asarray(x, np.float32)
    w_qkv = np.asarray(w_qkv, np.float32)
    features = np.asarray(features, np.float32)
    prototypes = np.asarray(prototypes, np.float32)
    theta = float(np.abs(np.asarray(theta, np.float32)))
    alpha = float(np.abs(np.asarray(alpha, np.float32)))
    beta = float(np.abs(np.asarray(beta, np.float32)))
    q_gain = np.asarray(q_gain, np.float32)
    lam = np.asarray(diff_lambda, np.float32)

    w_deq = _ternary_deq(w_qkv)
    p_deq = _ternary_deq(prototypes)
    cosT, sinT = _rope_tables(S)       # [32, S]

    rows = np.arange(128)
    sgn = np.where((rows // 32) % 2 == 0, 1.0, -1.0).astype(np.float32)
    cosF = cosT[rows % 32, :].astype(BF)
    sinF = (sinT[rows % 32, :] * sgn[:, None]).astype(BF)

    # diag-block causal mask, tiled 4x horizontally: [128, 512]
    tri = (np.arange(128)[None, :] >= np.arange(128)[:, None]).astype(np.float32)
    tri4 = np.tile(tri, (1, 4)).astype(BF)

    # stats masks: [128, 32] = 4 tile-types x 8 stat rows
    mask8 = np.zeros((128, 32), np.float32)
    for tt in range(4):
        for r in range(128):
            head_in_tile = r // 64
            row = (tt // 2) * 4 + (tt % 2) * 2 + head_in_tile
            mask8[r, tt * 8 + row] = 1.0
    mask8 = mask8.astype(BF)

    # strip-sum mask: rows 32p+i (i<16) -> col i
    smask = np.zeros((128, 16), np.float32)
    for p in range(4):
        for i in range(16):
            smask[32 * p + i, i] = 1.0

    ident = np.eye(128, dtype=np.float32).astype(BF)

    in_maps = []
    for c in range(N_CORES):
        b, hg = c // 4, c % 4
        h0 = hg * HPC
        qrows = slice(h0 * HD, (h0 + HPC) * HD)
        o0 = hg * OSL

        wq = w_deq[0:DIM][qrows]                   # [256, 1024]
        wk = w_deq[DIM:2 * DIM][qrows]             # [256, 1024]
        wv = w_deq[2 * DIM:3 * DIM][qrows]         # [256, 1024]

        # vw fold: per (h, f) channel block [k,16] = x @ (wv_f^T W_hf^T)
        # W_h0 = M1 + M2, W_h1 = lam_h (M2 - M1), M = features[:, head dims]
        wfold = np.zeros((DIM, 8 * NF), np.float32)
        for h in range(HPC):
            gh = h0 + h
            M1 = features[:, gh * 64:gh * 64 + 32]       # [16, 32]
            M2 = features[:, gh * 64 + 32:gh * 64 + 64]
            Wh0 = (M1 + M2)                               # [16, 32]
            Wh1 = lam[gh] * (M2 - M1)
            v0 = wv[h * 64:h * 64 + 32]                   # [32, 1024]
            v1 = wv[h * 64 + 32:h * 64 + 64]
            wfold[:, (2 * h) * NF:(2 * h + 1) * NF] = v0.T @ Wh0.T
            wfold[:, (2 * h + 1) * NF:(2 * h + 2) * NF] = v1.T @ Wh1.T

        wT = np.concatenate([wq.T, wk.T, wfold], axis=1)  # [1024, 640]

        gains = q_gain[h0:h0 + HPC] / math.sqrt(HD // 2)
        assert np.all(gains > 0), "nonpositive q_gain unsupported by ln-fold"
        lngain = np.zeros((8, 1), np.float32)
        lngain[0:4, 0] = np.log(gains)

        p_f = p_deq[o0:o0 + OSL] @ features.T          # [256, 16]
        p_s = _sigmoid(5.0 * p_f)
        p_a = p_f * p_s
        A_eff = theta * p_a - alpha * (1.0 - p_s)
        B_eff = -beta * p_a

        m = {
            "xT": np.ascontiguousarray(x[b].T).astype(BF),
            "wT": np.ascontiguousarray(wT).astype(BF),
            "cosF": cosF, "sinF": sinF,
            "tri4": tri4, "mask8": mask8,
            "smask": smask.astype(BF),
            "lngain": lngain,
            "ident": ident,
            "AT": np.ascontiguousarray(A_eff.T).astype(BF),
            "BT": np.ascontiguousarray(B_eff.T).astype(BF),
        }
        in_maps.append(m)
    return in_maps


def assemble_output(S, results):
    out = np.empty((B, S, DIM), np.float32)
    for c in range(N_CORES):
        b, hg = c // 4, c % 4
        out[b, :, hg * OSL:(hg + 1) * OSL] = results[c]["out"].T
    return out


_PROGRAM_CACHE = {}


def kernel(x, w_qkv, features, prototypes, theta, alpha, beta, q_gain,
           diff_lambda, _trace=False):
    x = np.asarray(x, np.float32)
    S = x.shape[1]
    if S not in _PROGRAM_CACHE:
        _PROGRAM_CACHE[S] = build_program(S)
    nc = _PROGRAM_CACHE[S]

    in_maps = make_in_maps(S, x, w_qkv, features, prototypes, theta, alpha,
                           beta, q_gain, diff_lambda)
    res = run_bass_kernel_spmd(nc, in_maps, list(range(N_CORES)),
                               trace=_trace)
    out = assemble_output(S, res.results)
    if _trace:
        return out, res
    return out


# revision 35
# speedup vs baseline: 2.0714x; 1.0531x over previous
"""Trainium2 Bass kernel for nn_CausalSelfAttention_68496138437292.

Sharding: 8 cores = 2 batches x 4 head-groups; core c handles batch c//4 and
local heads [4*(c%4), 4*(c%4)+4).  The Tversky projection is sharded over
out_features (each core computes a 256-wide o-slice); the 16-wide feature
contraction x_f (summed over D, split across head-groups) uses one small
AllReduce per 512-token chunk over each batch's 4-core group.

Key structural ideas (vs the phase-serial f32r baseline):
- bf16 everywhere on the PE; all (head, half) channel blocks are packed 4 per
  128-partition tile in natural order, so projection PSUM drains straight into
  the attention layout with no DMA repacking.
- The attention OUTPUT is never materialized: the final output only needs
  x_f = feat . y^T, and  feat_slice . (P V / den)^T = (P (V W^T))^T / den,
  with  V W^T = x @ (w_v^T W^T)  folded into the projection as 128 extra
  channels (host-precomputed fold).  Attention per (head, half) reduces to
  scores -> exp -> one 17-wide PV matmul (16 feat cols + a ones column that
  emits the softmax denominator for free).
- Scores/PV use PE array tiling (tile_position) for ~1.4x matmul throughput;
  exp runs as [128, 1024] ACT ops over multi-bank PSUM with trimmed causal
  widths; rmsnorm rsqrt+gain is one ln + one exp (single ACT table set).
- Projection of chunk c+1 is interleaved into attention of chunk j=c so the
  PE keeps busy while ACT chews exp; Tversky tail is pipelined per chunk with
  one [16, 512] AllReduce each.
"""

import math
from contextlib import ExitStack

import ml_dtypes
import numpy as np

import concourse.bass as bass
import concourse.mybir as mybir
import concourse.tile as tile
from concourse import bacc
from concourse.bass_utils import run_bass_kernel_spmd

F32 = mybir.dt.float32
F32R = mybir.dt.float32r
BF16 = mybir.dt.bfloat16
AF = mybir.ActivationFunctionType
ALU = mybir.AluOpType

DIM, NH, HD = 1024, 16, 64
ROPE_BASE, TRAIN_LEN, YARN_MAX = 10000.0, 1024, 4096
GROUP = 64
EPS = 1e-05
B = 2
N_CORES = 8
HPC = 4          # heads per core
OSL = 256        # out-feature slice per core
NF = 16          # tversky feature count
PVW = 17         # PV rhs width: 16 feat cols + ones col

BF = ml_dtypes.bfloat16


# ----------------------------------------------------------------- host math

def _ternary_deq(w: np.ndarray) -> np.ndarray:
    bf = ml_dtypes.bfloat16
    wb = w.astype(bf)
    wg = wb.reshape(-1, GROUP)
    m = (np.sum(np.abs(wg), axis=-1, keepdims=True, dtype=np.float32) / GROUP).astype(bf)
    scale = np.maximum(m.astype(np.float32), np.float32(1e-8)).astype(bf)
    ratio = (wg.astype(np.float32) / scale.astype(np.float32)).astype(bf)
    q = np.clip(np.round(ratio.astype(np.float32)), -1.0, 1.0).astype(bf)
    deq = (q.astype(np.float32) * scale.astype(np.float32)).astype(bf)
    return deq.reshape(wb.shape).astype(np.float32)


def _rope_tables(seqlen: int):
    rd = HD
    ar = np.arange(0, rd, 2, dtype=np.float32)
    inv_freq = 1.0 / ROPE_BASE ** (ar / rd)
    scale = TRAIN_LEN / YARN_MAX
    ramp = np.clip((ar / rd - 0.25) / 0.75, 0.0, 1.0)
    inv_freq = inv_freq / (ramp * (1.0 / scale - 1.0) + 1.0)
    freqs = np.arange(seqlen, dtype=np.float32)[:, None] * inv_freq[None, :]
    return np.cos(freqs).T.astype(np.float32), np.sin(freqs).T.astype(np.float32)


def _sigmoid(x):
    return 1.0 / (1.0 + np.exp(-x))


# ------------------------------------------------------------ device program

def build_program(S: int, dbg: bool = False):
    NC = S // 512            # 512-token chunks
    NT = S // 128            # 128-token k tiles

    nc = bacc.Bacc("TRN2", target_bir_lowering=False, debug=False,
                   num_devices=N_CORES)

    # DRAM I/O
    d_xT = nc.dram_tensor("xT", [DIM, S], BF16, kind="ExternalInput")
    # 640 = 256 q + 256 k + 128 vw-fold channels
    d_wT = nc.dram_tensor("wT", [DIM, 640], BF16, kind="ExternalInput")
    d_cosF = nc.dram_tensor("cosF", [128, S], BF16, kind="ExternalInput")
    d_sinF = nc.dram_tensor("sinF", [128, S], BF16, kind="ExternalInput")
    d_tri4 = nc.dram_tensor("tri4", [128, 512], BF16, kind="ExternalInput")
    d_mask8 = nc.dram_tensor("mask8", [128, 32], BF16, kind="ExternalInput")
    d_smask = nc.dram_tensor("smask", [128, 16], BF16, kind="ExternalInput")
    d_lngain = nc.dram_tensor("lngain", [8, 1], F32, kind="ExternalInput")
    d_ident = nc.dram_tensor("ident", [128, 128], BF16, kind="ExternalInput")
    d_AT = nc.dram_tensor("AT", [16, OSL], BF16, kind="ExternalInput")
    d_BT = nc.dram_tensor("BT", [16, OSL], BF16, kind="ExternalInput")
    d_out = nc.dram_tensor("out", [OSL, S], F32, kind="ExternalOutput")
    if dbg:
        d_dbg_xf = nc.dram_tensor("dbg_xf", [16, S], F32, kind="ExternalOutput")
        d_dbg_xfar = nc.dram_tensor("dbg_xfar", [16, S], F32, kind="ExternalOutput")

    with tile.TileContext(nc) as tc:
        persist = ExitStack()
        cpool = persist.enter_context(tc.tile_pool(name="consts", bufs=1))
        qkpool = persist.enter_context(tc.tile_pool(name="qk", bufs=1))
        vwpool = persist.enter_context(tc.tile_pool(name="vwrhs", bufs=1))
        xfpool = persist.enter_context(tc.tile_pool(name="xft", bufs=1))
        wpool = persist.enter_context(tc.tile_pool(name="wts", bufs=1))
        drpool = persist.enter_context(
            tc.tile_pool(name="drscratch", bufs=1, space="DRAM"))

        # transient pools
        xpool = persist.enter_context(tc.tile_pool(name="xstream", bufs=8))
        sqpool = persist.enter_context(tc.tile_pool(name="sq", bufs=4))
        scpool = persist.enter_context(tc.tile_pool(name="scb", bufs=4))
        s8pool = persist.enter_context(tc.tile_pool(name="sc8", bufs=2))
        rppool = persist.enter_context(tc.tile_pool(name="ropetmp", bufs=2))
        espool = persist.enter_context(tc.tile_pool(name="es", bufs=6))
        tailp = persist.enter_context(tc.tile_pool(name="tail", bufs=2))

        # PSUM budget (8 banks): proj 1 + aux 1 + waves 2x2 + pv 2 = 8
        proj_ps = persist.enter_context(
            tc.tile_pool(name="proj", bufs=1, space="PSUM"))
        aux_ps = persist.enter_context(
            tc.tile_pool(name="aux", bufs=1, space="PSUM"))
        wave_ps = persist.enter_context(
            tc.tile_pool(name="wave", bufs=2, space="PSUM"))
        pv_ps = persist.enter_context(
            tc.tile_pool(name="pv", bufs=1, space="PSUM"))

        def aux_tile():
            # single shared [128, 512] psum bank; callers slice what they need
            return aux_ps.tile([128, 512], F32, tag="aux", name="aux")

        # ---- persistent SBUF ----
        # weights first: the first projection chain only needs wts + x(0)
        wts0 = wpool.tile([128, 640], BF16, name="w0")
        nc.sync.dma_start(wts0[:], d_wT[0:128, :])
        cosF = cpool.tile([128, S], BF16, name="cosF")
        sinF = cpool.tile([128, S], BF16, name="sinF")
        tri4 = cpool.tile([128, 512], BF16, name="tri4")
        mask8 = cpool.tile([128, 32], BF16, name="mask8")
        smask = cpool.tile([128, 16], BF16, name="smask")
        lngain = cpool.tile([8, 1], F32, name="lngain")
        eps8 = cpool.tile([8, 1], F32, name="eps8")
        nc.vector.memset(eps8[:], EPS)
        ident = cpool.tile([128, 128], BF16, name="ident")
        AT = cpool.tile([16, OSL], BF16, name="AT")
        BT = cpool.tile([16, OSL], BF16, name="BT")

        # Pre-load the exp+ln table set so the placement pass never needs to
        # thrash between exp_and_others / natural_log per chunk.
        tables = list(__import__("concourse.hw_specs", fromlist=["x"])
                      .get_activation_tables(nc.m.arch).keys())
        set_id = tables.index("natural_log_exp_and_others")
        nc.scalar.add_instruction(mybir.InstLoadActFuncSet(
            name=nc.get_next_instruction_name(), act_func_set_id=set_id,
            ins=[], outs=[]))

        wts = [wts0] + [wpool.tile([128, 640], BF16, name=f"w{d}")
                        for d in range(1, 8)]
        _wq = [nc.sync, nc.gpsimd, nc.sync]
        for d in range(1, 8):
            _wq[d % 3].dma_start(wts[d][:], d_wT[d * 128:(d + 1) * 128, :])
        # consts go on the gpsimd DMA queue so they don't delay weights/x
        nc.gpsimd.dma_start(cosF[:], d_cosF[:])
        nc.gpsimd.dma_start(sinF[:], d_sinF[:])
        nc.gpsimd.dma_start(tri4[:], d_tri4[:])
        nc.gpsimd.dma_start(mask8[:], d_mask8[:])
        nc.gpsimd.dma_start(smask[:], d_smask[:])
        nc.gpsimd.dma_start(lngain[:], d_lngain[:])
        nc.gpsimd.dma_start(ident[:], d_ident[:])
        nc.gpsimd.dma_start(AT[:], d_AT[:])
        nc.gpsimd.dma_start(BT[:], d_BT[:])

        qa = [qkpool.tile([128, S], BF16, name=f"qa{t}") for t in range(2)]
        ka = [qkpool.tile([128, S], BF16, name=f"ka{t}") for t in range(2)]
        # PV rhs per ktile: [k 128, 8 problems x 17]; ones col at 16 mod 17
        rhs_vw = [vwpool.tile([128, 8 * PVW], BF16, name=f"rvw{t}")
                  for t in range(NT)]
        for t in range(NT):
            nc.vector.memset(
                rhs_vw[t][:].rearrange("p (g c) -> p g c", c=PVW)[:, :, 16:17],
                1.0)
        # xf^T strips per tile-group: rows 32p..32p+16 = (xf contrib | den)
        xft = [xfpool.tile([128, S], F32, name=f"xft{t}") for t in range(2)]

        # ---------------- emission helpers ----------------

        def proj_chunk_steps(c):
            """Returns a list of closures emitting projection of chunk c."""
            s0 = c * 512
            steps = []
            xt = [None] * 8
            sq_t = [None] * 4
            stat = [None]
            sc8 = [None]

            def load_x():
                for d in range(8):
                    xt[d] = xpool.tile([128, 512], BF16, tag="xt", name="xt")
                    nc.sync.dma_start(xt[d][:], d_xT[d * 128:(d + 1) * 128,
                                                     s0:s0 + 512])
            steps.append(load_x)

            # 4 qk chains: ot 0,1 = q tiles, ot 2,3 = k tiles
            def make_qk(ot):
                def f():
                    dst = qa[ot] if ot < 2 else ka[ot - 2]
                    pq = proj_ps.tile([128, 512], F32, tag="pmm", name="pmm")
                    for d in range(8):
                        nc.tensor.matmul(pq[:], wts[d][:, ot * 128:(ot + 1) * 128],
                                         xt[d][:], start=(d == 0), stop=(d == 7))
                    # unscaled drain (scale applied later in-place)
                    nc.vector.tensor_copy(dst[:, s0:s0 + 512], pq[:])
                    sq = sqpool.tile([128, 512], BF16, tag="sq", name="sq")
                    nc.vector.tensor_mul(sq[:], dst[:, s0:s0 + 512],
                                         dst[:, s0:s0 + 512])
                    sq_t[ot] = sq
                return f
            for ot in range(4):
                steps.append(make_qk(ot))

            def stats():
                st = aux_tile()
                for ot in range(4):
                    nc.tensor.matmul(st[0:8, :], mask8[:, ot * 8:(ot + 1) * 8],
                                     sq_t[ot][:], start=(ot == 0), stop=(ot == 3))
                lnt = s8pool.tile([8, 512], F32, tag="lnt", name="lnt")
                nc.scalar.activation(lnt[:], st[0:8, :], AF.Ln, scale=1.0 / HD,
                                     bias=eps8[:])
                s8 = s8pool.tile([8, 512], BF16, tag="sc8", name="sc8")
                nc.scalar.activation(s8[:], lnt[:], AF.Exp, scale=-0.5,
                                     bias=lngain[:])
                sc8[0] = s8
            steps.append(stats)

            def scale_rope():
                # broadcast row scales into block layout (via DRAM scratch --
                # SBUF sources cannot have a zero partition step), then
                # scale+rope
                dr8 = drpool.tile([8, 512], BF16, tag="dr8", bufs=2, name="dr8")
                nc.sync.dma_start(dr8[:], sc8[0][:])
                qs = [nc.sync, nc.gpsimd, nc.sync, nc.gpsimd]
                scb = []
                for tt in range(4):          # 2 q tiles then 2 k tiles
                    sb = scpool.tile([128, 512], BF16, tag="scb", name="scb")
                    for hh in range(2):
                        row = (tt // 2) * 4 + (tt % 2) * 2 + hh
                        qs[tt].dma_start(
                            sb[hh * 64:hh * 64 + 64, :],
                            dr8[row:row + 1, :].to_broadcast([64, 512]))
                    scb.append(sb)
                tiles = [qa[0], qa[1], ka[0], ka[1]]
                for tt in range(4):
                    nc.vector.tensor_mul(tiles[tt][:, s0:s0 + 512],
                                         tiles[tt][:, s0:s0 + 512], scb[tt][:])
                # rope: prefetch partner blocks (adjacent 32-row block), then
                # x = x*cos + prt*sinF (sinF carries the half sign)
                for tt in range(4):
                    prt = rppool.tile([128, 512], BF16, tag="prt", name="prt")
                    for p in range(4):
                        qs[(tt + p) % 3].dma_start(
                            prt[32 * p:32 * p + 32, :],
                            tiles[tt][32 * (p ^ 1):32 * (p ^ 1) + 32, s0:s0 + 512])
                    tb = rppool.tile([128, 512], BF16, tag="tb", name="tb")
                    nc.vector.tensor_mul(tb[:], prt[:], sinF[:, s0:s0 + 512])
                    nc.vector.tensor_mul(tiles[tt][:, s0:s0 + 512],
                                         tiles[tt][:, s0:s0 + 512],
                                         cosF[:, s0:s0 + 512])
                    nc.vector.tensor_add(tiles[tt][:, s0:s0 + 512],
                                         tiles[tt][:, s0:s0 + 512], tb[:])
            steps.append(scale_rope)

            def vw_chain():
                pv = proj_ps.tile([128, 512], F32, tag="pmm", name="pmm")
                for d in range(8):
                    nc.tensor.matmul(pv[:], wts[d][:, 512:640], xt[d][:],
                                     start=(d == 0), stop=(d == 7))
                vw_sb = sqpool.tile([128, 512], BF16, tag="vwsb", name="vwsb")
                nc.vector.tensor_copy(vw_sb[:], pv[:])
                # transpose each 128-token block: vw_sb [ch 128, s] -> [s, ch]
                for i in range(4):
                    pt = aux_tile()
                    nc.tensor.matmul(pt[:, 0:128], vw_sb[:, i * 128:(i + 1) * 128],
                                     ident[:], start=True, stop=True)
                    t = c * 4 + i
                    dst = rhs_vw[t][:].rearrange(
                        "p (g c) -> p g c", c=PVW)[:, :, 0:16]
                    nc.vector.tensor_copy(
                        dst, pt[:, 0:128].rearrange("p (g c) -> p g c", c=16))
            steps.append(vw_chain)
            return steps

        # attention state: per chunk j, accumulate xf strips in 2 psum banks
        def attn_chunk(j, inject):
            """Emit attention for q chunk j; call inject() between t-steps to
            interleave next chunk's projection work."""
            ntk = 4 * (j + 1)
            xfa = [pv_ps.tile([128, 512], F32, tag=f"xfa{qt}", name=f"xfa{qt}")
                   for qt in range(2)]
            es_q = {}
            LAG = 2

            def emit_pv(t):
                off = max(0, (t - 4 * j) * 128)
                for qt in range(2):
                    es = es_q.pop((t, qt))
                    es3 = es[:].rearrange("p (g c) -> p g c", c=512)
                    for p in range(4):
                        nc.tensor.matmul(
                            xfa[qt][32 * p:32 * p + PVW, off:512],
                            rhs_vw[t][:, (4 * qt + p) * PVW:(4 * qt + p + 1) * PVW],
                            es3[:, p, off:512],
                            start=(t == 0), stop=(t == ntk - 1),
                            tile_position=(0, 32 * p),
                            skip_group_check=True)

            for t in range(ntk):
                off = max(0, (t - 4 * j) * 128)
                w = 512 - off
                for qt in range(2):
                    es = espool.tile([128, 2048], BF16, tag="es", name="es")
                    for pair in range(2):
                        ps = wave_ps.tile([128, 1024], F32, tag="wv", name="wv")
                        for pp in range(2):
                            p = pair * 2 + pp
                            nc.tensor.matmul(
                                ps[:, pp * 512 + off:(pp + 1) * 512],
                                ka[qt][32 * p:32 * p + 32, t * 128:(t + 1) * 128],
                                qa[qt][32 * p:32 * p + 32,
                                       j * 512 + off:(j + 1) * 512],
                                start=True, stop=True,
                                tile_position=(32 * p, 0))
                        ps3 = ps[:].rearrange("p (g c) -> p g c", c=512)
                        es3 = es[:].rearrange("p (g c) -> p g c", c=512)
                        nc.scalar.activation(
                            es3[:, 2 * pair:2 * pair + 2, off:512],
                            ps3[:, :, off:512], AF.Exp)
                    if t >= 4 * j:
                        # causal mask on the diagonal 128-block of each strip
                        dv = es[:].rearrange("p (g c) -> p g c", c=512)[
                            :, :, off:off + 128]
                        tri = tri4[:].rearrange("p (g c) -> p g c", c=128)
                        nc.vector.tensor_mul(dv, dv, tri)
                    es_q[(t, qt)] = es
                if t >= LAG:
                    emit_pv(t - LAG)
                inject()
            for t in range(max(0, ntk - LAG), ntk):
                emit_pv(t)

            # drain strips to SBUF
            for qt in range(2):
                nc.vector.tensor_copy(xft[qt][:, j * 512:(j + 1) * 512],
                                      xfa[qt][:])

        # ---------------- tversky tail (per chunk) ----------------
        cc_in = [drpool.tile([16, 512], F32, name=f"ccin{j}") for j in range(NC)]
        cc_out = [drpool.tile([16, 512], F32, name=f"ccout{j}") for j in range(NC)]

        def tail_chunk_steps(j):
            s0 = j * 512
            st = {}

            def t1_dens():
                # gather dens rows (strip row 16 of each 32-block), then
                # reciprocal = exp(-ln) on ACT (set stays resident), stage
                # to DRAM for the partition-broadcast
                dens = tailp.tile([8, 512], F32, tag="dens", name="dens")
                dq = [nc.gpsimd, nc.sync, nc.gpsimd, nc.sync]
                for qt in range(2):
                    for p in range(4):
                        dq[p].dma_start(
                            dens[qt * 4 + p:qt * 4 + p + 1, :],
                            xft[qt][32 * p + 16:32 * p + 17, s0:s0 + 512])
                lnd = tailp.tile([8, 512], F32, tag="lnd", name="lnd")
                nc.scalar.activation(lnd[:], dens[:], AF.Ln)
                rb = tailp.tile([8, 512], F32, tag="rb", name="rb")
                nc.scalar.activation(rb[:], lnd[:], AF.Exp, scale=-1.0)
                drb = drpool.tile([8, 512], F32, tag="drb", bufs=2, name="drb")
                nc.sync.dma_start(drb[:], rb[:])
                st["drb"] = drb

            def t2_xf():
                drb = st["drb"]
                rbb = [tailp.tile([128, 512], F32, tag=f"rbb{qt}", name="rbb")
                       for qt in range(2)]
                dq = [nc.gpsimd, nc.sync, nc.gpsimd, nc.sync]
                for qt in range(2):
                    for p in range(4):
                        dq[p].dma_start(
                            rbb[qt][32 * p:32 * p + 16, :],
                            drb[qt * 4 + p:qt * 4 + p + 1, :]
                            .to_broadcast([16, 512]))
                sc = [tailp.tile([128, 512], BF16, tag=f"sc{qt}", name="sc")
                      for qt in range(2)]
                for qt in range(2):
                    nc.vector.tensor_mul(sc[qt][:],
                                         xft[qt][:, s0:s0 + 512], rbb[qt][:])
                pxf = aux_tile()
                for qt in range(2):
                    nc.tensor.matmul(pxf[0:16, :], smask[:], sc[qt][:],
                                     start=(qt == 0), stop=(qt == 1))
                xfl = tailp.tile([16, 512], F32, tag="xfl", name="xfl")
                nc.vector.tensor_copy(xfl[:], pxf[0:16, :])
                if dbg:
                    nc.sync.dma_start(d_dbg_xf[:, s0:s0 + 512], xfl[:])
                nc.sync.dma_start(cc_in[j][:], xfl[:])
                nc.gpsimd.collective_compute(
                    "AllReduce", ALU.add,
                    replica_groups=[[0, 1, 2, 3], [4, 5, 6, 7]],
                    ins=[cc_in[j][:]], outs=[cc_out[j][:]])

            def t3_out():
                xf = tailp.tile([16, 512], F32, tag="xfr", name="xfr")
                nc.sync.dma_start(xf[:], cc_out[j][:])
                if dbg:
                    nc.sync.dma_start(d_dbg_xfar[:, s0:s0 + 512], xf[:])
                # xa = xf*sig(5xf) = xf/(1+e), oms = e/(1+e), e = exp(-5 xf)
                e = tailp.tile([16, 512], F32, tag="e", name="e")
                nc.scalar.activation(e[:], xf[:], AF.Exp, scale=-5.0)
                t1 = tailp.tile([16, 512], F32, tag="t1", name="t1")
                nc.vector.tensor_scalar_add(t1[:], e[:], 1.0)
                lt = tailp.tile([16, 512], F32, tag="lt", name="lt")
                nc.scalar.activation(lt[:], t1[:], AF.Ln)
                r = tailp.tile([16, 512], F32, tag="r", name="r")
                nc.scalar.activation(r[:], lt[:], AF.Exp, scale=-1.0)
                xa = tailp.tile([16, 512], BF16, tag="xa", name="xa")
                oms = tailp.tile([16, 512], BF16, tag="oms", name="oms")
                nc.vector.tensor_mul(xa[:], xf[:], r[:])
                nc.vector.tensor_mul(oms[:], e[:], r[:])
                for ot in range(2):
                    po = aux_tile()
                    nc.tensor.matmul(po[:], AT[:, ot * 128:(ot + 1) * 128],
                                     xa[:], start=True, stop=False)
                    nc.tensor.matmul(po[:], BT[:, ot * 128:(ot + 1) * 128],
                                     oms[:], start=False, stop=True)
                    ob = tailp.tile([128, 512], F32, tag="ob", name="ob")
                    nc.vector.tensor_copy(ob[:], po[:])
                    nc.sync.dma_start(
                        d_out[ot * 128:(ot + 1) * 128, s0:s0 + 512], ob[:])

            return [t1_dens, t2_xf, t3_out]

        # ---------------- main schedule ----------------
        # attn(j) runs with proj(j+1) and tail(j-1) steps injected between
        # t-iterations so the PE never sits behind a long-latency tail chain.
        for step in proj_chunk_steps(0):
            step()
        for j in range(NC):
            pending = []
            if j >= 1:
                pending += tail_chunk_steps(j - 1)
            if j + 1 < NC:
                # interleave proj steps after the tail kickoff steps
                pending += proj_chunk_steps(j + 1)
            counter = [0]
            # front-load: finish all injected steps by ~45% of the t loop so
            # the next chunk's q/k are roped before this chunk's waves end
            horizon = max(1, int(4 * (j + 1) * 0.45))
            nsteps = len(pending)

            def inject():
                counter[0] += 1
                want = min(nsteps, -(-counter[0] * nsteps // horizon))
                while len(pending) and (nsteps - len(pending)) < want:
                    pending.pop(0)()
            attn_chunk(j, inject)
            while pending:
                pending.pop(0)()
        for step in tail_chunk_steps(NC - 1):
            step()

        persist.close()

    nc.compile()
    return nc


# ----------------------------------------------------------- host marshaling

def make_in_maps(S, x, w_qkv, features, prototypes, theta, alpha, beta,
                 q_gain, diff_lambda):
    x = np.asarray(x, np.float32)
    w_qkv = np.asarray(w_qkv, np.float32)
    features = np.asarray(features, np.float32)
    prototypes = np.asarray(prototypes, np.float32)
    theta = float(np.abs(np.asarray(theta, np.float32)))
    alpha = float(np.abs(np.asarray(alpha, np.float32)))
    beta = float(np.abs(np.asarray(beta, np.float32)))
    q_gain = np.asarray(q_gain, np.float32)
    lam = np.asarray(diff_lambda, np.float32)

    w_deq = _ternary_deq(w_qkv)
    p_deq = _ternary_deq(prototypes)
    cosT, sinT = _rope_tables(S)       # [32, S]

    rows = np.arange(128)
    sgn = np.where((rows // 32) % 2 == 0, 1.0, -1.0).astype(np.float32)
    cosF = cosT[rows % 32, :].astype(BF)
    sinF = (sinT[rows % 32, :] * sgn[:, None]).astype(BF)

    # diag-block causal mask, tiled 4x horizontally: [128, 512]
    tri = (np.arange(128)[None, :] >= np.arange(128)[:, None]).astype(np.float32)
    tri4 = np.tile(tri, (1, 4)).astype(BF)

    # stats masks: [128, 32] = 4 tile-types x 8 stat rows
    mask8 = np.zeros((128, 32), np.float32)
    for tt in range(4):
        for r in range(128):
            head_in_tile = r // 64
            row = (tt // 2) * 4 + (tt % 2) * 2 + head_in_tile
            mask8[r, tt * 8 + row] = 1.0
    mask8 = mask8.astype(BF)

    # strip-sum mask: rows 32p+i (i<16) -> col i
    smask = np.zeros((128, 16), np.float32)
    for p in range(4):
        for i in range(16):
            smask[32 * p + i, i] = 1.0

    ident = np.eye(128, dtype=np.float32).astype(BF)

    in_maps = []
    for c in range(N_CORES):
        b, hg = c // 4, c % 4
        h0 = hg * HPC
        qrows = slice(h0 * HD, (h0 + HPC) * HD)
        o0 = hg * OSL

        wq = w_deq[0:DIM][qrows]                   # [256, 1024]
        wk = w_deq[DIM:2 * DIM][qrows]             # [256, 1024]
        wv = w_deq[2 * DIM:3 * DIM][qrows]         # [256, 1024]

        # vw fold: per (h, f) channel block [k,16] = x @ (wv_f^T W_hf^T)
        # W_h0 = M1 + M2, W_h1 = lam_h (M2 - M1), M = features[:, head dims]
        wfold = np.zeros((DIM, 8 * NF), np.float32)
        for h in range(HPC):
            gh = h0 + h
            M1 = features[:, gh * 64:gh * 64 + 32]       # [16, 32]
            M2 = features[:, gh * 64 + 32:gh * 64 + 64]
            Wh0 = (M1 + M2)                               # [16, 32]
            Wh1 = lam[gh] * (M2 - M1)
            v0 = wv[h * 64:h * 64 + 32]                   # [32, 1024]
            v1 = wv[h * 64 + 32:h * 64 + 64]
            wfold[:, (2 * h) * NF:(2 * h + 1) * NF] = v0.T @ Wh0.T
            wfold[:, (2 * h + 1) * NF:(2 * h + 2) * NF] = v1.T @ Wh1.T

        wT = np.concatenate([wq.T, wk.T, wfold], axis=1)  # [1024, 640]

        gains = q_gain[h0:h0 + HPC] / math.sqrt(HD // 2)
        assert np.all(gains > 0), "nonpositive q_gain unsupported by ln-fold"
        lngain = np.zeros((8, 1), np.float32)
        lngain[0:4, 0] = np.log(gains)

        p_f = p_deq[o0:o0 + OSL] @ features.T          # [256, 16]
        p_s = _sigmoid(5.0 * p_f)
        p_a = p_f * p_s
        A_eff = theta * p_a - alpha * (1.0 - p_s)
        B_eff = -beta * p_a

        m = {
            "xT": np.ascontiguousarray(x[b].T).astype(BF),
            "wT": np.ascontiguousarray(wT).astype(BF),
            "cosF": cosF, "sinF": sinF,
            "tri4": tri4, "mask8": mask8,
            "smask": smask.astype(BF),
            "lngain": lngain,
            "ident": ident,
            "AT": np.ascontiguousarray(A_eff.T).astype(BF),
            "BT": np.ascontiguousarray(B_eff.T).astype(BF),
        }
        in_maps.append(m)
    return in_maps


def assemble_output(S, results):
    out = np.empty((B, S, DIM), np.float32)
    for c in range(N_CORES):
        b, hg = c // 4, c % 4
        out[b, :, hg * OSL:(hg + 1) * OSL] = results[c]["out"].T
    return out


_PROGRAM_CACHE = {}


def kernel(x, w_qkv, features, prototypes, theta, alpha, beta, q_gain,
           diff_lambda, _trace=False):
    x = np.asarray(x, np.float32)
    S = x.shape[1]
    if S not in _PROGRAM_CACHE:
        _PROGRAM_CACHE[S] = build_program(S)
    nc = _PROGRAM_CACHE[S]

    in_maps = make_in_maps(S, x, w_qkv, features, prototypes, theta, alpha,
                           beta, q_gain, diff_lambda)
    res = run_bass_kernel_spmd(nc, in_maps, list(range(N_CORES)),
                               trace=_trace)
    out = assemble_output(S, res.results)
    if _trace:
        return out, res
    return out
